# revision 1
# baseline (speedup 1.0000x reference)
"""EquiMHA Trainium2 kernel.

Data-parallel over batch B=8 across the 8 NeuronCores (one batch element per
core, weights replicated, no collectives).

Per-core computation for batch b (N=512, M=4, KN=512, DQ=DK=512, DV=1024,
H=16, D=64):
  Qp = Q[b] @ w_q, Kp = K[b] @ w_k, Vp = K[b] @ w_v
  E[h,n,k] = sum_{m,d} Qp[n,m,h*64+d] Kp[k,m,h*64+d] / 32
  A = masked_softmax(E)        (max-subtraction skipped: |E|/32 <= ~2, and the
                                max cancels exactly up to the +eps term)
  O[n,m,h*64+d] = sum_k A[h,n,k] Vp[k,m,h*64+d]
  out = O @ w_o

Precision strategy (tolerance is 2e-2; measured pipeline error ~8e-3):
  - Q/K-side projections run in fp8e4m3 with DoubleRow perf mode (2x PE
    rate, 256-deep contraction per pass). Host pre-quantizes Q, K and
    w_q, w_k (weights pre-scaled by 64 into fp8 normal range; the 64*64
    factor is folded into the exp scale).
  - Everything downstream (E scores, A@V, O@w_o) runs in bf16, which is
    full PE rate and halves SBUF/DMA vs f32 so the whole working set
    stays on-chip (no DRAM round trips).

Layout strategy: the host pre-transposes all inputs into the exact tile
layouts the PE wants, so the kernel does zero on-chip transposes:
  - QT8/KT8 [256,2,M,N]: [c*128+p, i, m, n] with dq = c*256+i*128+p, the
    DoubleRow pairing of two 128-deep contraction chunks.
  - P1/P2 emit qpp/kpp[h] = [(s,d) 128, mp, n|k] bf16 via partition-shifted
    psum evacuation (DoubleRow outputs land on psum partitions 0-63).
  - E^T[k,n] per head is a 2-matmul bf16 contraction over mp; softmax runs
    in [k, n] orientation entirely OFF the PE: exp on ACT (bf16 out), mask
    multiply + a bf16 partial-sum tree on DVE (2-byte fast mode), a Pool
    partition_all_reduce for the column sums (output already broadcast
    across partitions), and a bf16 DVE reciprocal.
  - Vp2[kc] = [128 k, (h, mp, s, d) 4096] bf16 so the O matmul stationary
    is one contiguous 128-column slice; O psum is normalized on DVE
    directly into OT[hp] = [(h%2,d), m, n] bf16 tiles, which are exactly
    the stationary operands P7 wants. Output leaves in natural [n, m, dvo]
    orientation for contiguous stores.
"""

import numpy as np
import ml_dtypes

import concourse.bacc as bacc
import concourse.mybir as mybir
import concourse.tile as tile

F32 = mybir.dt.float32
F32R = mybir.dt.float32r
F8 = mybir.dt.float8e4
BF = mybir.dt.bfloat16
AF = mybir.ActivationFunctionType
DR = mybir.MatmulPerfMode.DoubleRow

NPF8 = ml_dtypes.float8_e4m3
NPBF = ml_dtypes.bfloat16

B, N, M, KN = 8, 512, 4, 512
DQ, DK, DV, H = 512, 512, 1024, 16
D = DV // H
HP = H // 2          # head pairs (P7 contraction chunks)
KC = KN // 128       # k chunks
NC = N // 128        # n chunks
WS = 64.0            # host pre-scale for fp8 weights
SCALE = 1.0 / 32.0   # 1/sqrt(DV)
ESCALE = SCALE / (WS * WS)  # fused into exp


def build_nc():
    nc = bacc.Bacc("TRN2", target_bir_lowering=False, debug=False, num_devices=8)

    qt8_d = nc.dram_tensor("QT8", [128, 2, 2, M, N], F8, kind="ExternalInput")
    kt8_d = nc.dram_tensor("KT8", [128, 2, 2, M, KN], F8, kind="ExternalInput")
    ktb_d = nc.dram_tensor("KTB", [128, 4, M, KN], BF, kind="ExternalInput")
    mt_d = nc.dram_tensor("MT", [128, 2, 2, N], BF, kind="ExternalInput")
    wq8_d = nc.dram_tensor("WQ8", [128, 2, 2, DV], F8, kind="ExternalInput")
    wk8_d = nc.dram_tensor("WK8", [128, 2, 2, DV], F8, kind="ExternalInput")
    wvb_d = nc.dram_tensor("WVB", [128, 4, DV], BF, kind="ExternalInput")
    wob_d = nc.dram_tensor("WOB", [128, 8, DV], BF, kind="ExternalInput")
    out_d = nc.dram_tensor("out", [N, M, DV], F32, kind="ExternalOutput")

    def ecopy(eng, dst, src):
        if eng is nc.scalar:
            nc.scalar.copy(dst, src)
        else:
            eng.tensor_copy(dst, src)

    with tile.TileContext(nc) as tc:
        with tc.tile_pool(name="persist", bufs=1) as persist:
            mask2 = persist.tile([128, 2, 2, N], BF, name="mask2")
            vp2 = [persist.tile([128, M * DV], BF, name=f"vp2_{kc}") for kc in range(KC)]
            qpp = [persist.tile([128, 2, N], BF, name=f"qpp{h}") for h in range(H)]
            kpp = [persist.tile([128, 2, KN], BF, name=f"kpp{h}") for h in range(H)]
            ot = [persist.tile([128, M, N], BF, name=f"ot{hp}") for hp in range(HP)]
            wob = persist.tile([128, HP, DV], BF, name="wob")

            # ============ projections ============
            # Phase A: P2 (fp8 DR, evac-heavy) interleaved with half of P3
            # (bf16, PE-heavy); phase B: P1 interleaved with the other half.
            # Interleaving keeps the psum-evac engines (ACT/DVE) loaded
            # continuously instead of bursting past PE per phase.
            eng_ns = {"act": 0.0, "dve": 0.0}

            def pick_eng(act_cost, dve_cost):
                if eng_ns["act"] + act_cost <= eng_ns["dve"] + dve_cost:
                    eng_ns["act"] += act_cost
                    return nc.scalar
                eng_ns["dve"] += dve_cost
                return nc.vector

            with (
                tc.tile_pool(name="xk8", bufs=1) as xk8p,
                tc.tile_pool(name="w8", bufs=1) as w8p,
                tc.tile_pool(name="xkb", bufs=1) as xkbp,
                tc.tile_pool(name="wvb", bufs=1) as wvbp,
                tc.tile_pool(name="ppj", bufs=3, space="PSUM") as ppj,
                tc.tile_pool(name="ppv", bufs=2, space="PSUM") as ppv,
            ):
                # merged [p, c, ...] tiles: one DMA folds both 128-row
                # blocks of the 256-row dram tensors, minimizing the number
                # of serialized descriptor-generation setups on the SP queue
                xk8 = xk8p.tile([128, 2, 2, M, KN], F8, name="xk8")
                wk8 = w8p.tile([128, 2, 2, DV], F8, name="wk8")
                xkb = xkbp.tile([128, 4, M, KN], BF, name="xkb")
                wvb = wvbp.tile([128, 4, DV], BF, name="wvb")
                xq8 = xk8p.tile([128, 2, 2, M, N], F8, name="xq8")
                wq8 = w8p.tile([128, 2, 2, DV], F8, name="wq8")
                nc.sync.dma_start(wk8, wk8_d.ap())
                for m in (0, 2, 1, 3):
                    nc.sync.dma_start(
                        xk8[:, :, :, m, :], kt8_d.ap()[:, :, :, m, :]
                    )
                for c in range(4):
                    nc.sync.dma_start(wvb[:, c, :], wvb_d.ap()[:, c, :])
                    nc.sync.dma_start(xkb[:, c, :, :], ktb_d.ap()[:, c, :, :])
                nc.sync.dma_start(wq8, wq8_d.ap())
                nc.sync.dma_start(xq8, qt8_d.ap())
                nc.sync.dma_start(mask2, mt_d.ap())
                nc.sync.dma_start(wob, wob_d.ap())

                def proj8(h, s, w_sb, x_sb, dst):
                    pq2 = ppj.tile([64, 1024], F32, name="pq2", tag="pq2")
                    for mp in range(2):
                        m = 2 * mp + s
                        for nh in range(2):
                            for c in range(2):
                                nc.tensor.matmul(
                                    pq2[:, mp * 512 + nh * 256 : mp * 512 + (nh + 1) * 256],
                                    w_sb[:, c, :, h * 64 : (h + 1) * 64],
                                    x_sb[:, c, :, m, nh * 256 : (nh + 1) * 256],
                                    start=(c == 0),
                                    stop=(c == 1),
                                    perf_mode=DR,
                                )
                    ecopy(
                        pick_eng(1030, 1310),
                        dst[s * 64 : (s + 1) * 64, :, :],
                        pq2.rearrange("p (t n) -> p t n", t=2),
                    )

                p3_units = [
                    (mi, kc, dvh)
                    for mi in range(M)
                    for kc in range(KC)
                    for dvh in range(2)
                ]

                def p3_unit(u):
                    mi, kc, dvh = u
                    off = (mi // 2) * 128 + (mi % 2) * 64
                    pv = ppv.tile([128, 512], F32, name="pv", tag="pv")
                    for c in range(4):
                        nc.tensor.matmul(
                            pv,
                            xkb[:, c, mi, kc * 128 : (kc + 1) * 128],
                            wvb[:, c, dvh * 512 : (dvh + 1) * 512],
                            start=(c == 0),
                            stop=(c == 3),
                        )
                    v4 = vp2[kc].rearrange("p (h c) -> p h c", h=H)
                    ecopy(
                        pick_eng(610, 730),
                        v4[:, dvh * 8 : (dvh + 1) * 8, off : off + 64],
                        pv.rearrange("p (h d) -> p h d", h=8),
                    )

                p3i = 0
                # phase A: P2, s-major (the s=0 pass needs only half of K);
                # two P3 units per head in the second pass
                for h in range(H):
                    proj8(h, 0, wk8, xk8, kpp[h])
                for h in range(H):
                    proj8(h, 1, wk8, xk8, kpp[h])
                    if h >= 8:
                        p3_unit(p3_units[p3i]); p3i += 1
                        p3_unit(p3_units[p3i]); p3i += 1
                # phase B: P1 x16 heads; one P3 unit per head
                for h in range(H):
                    proj8(h, 0, wq8, xq8, qpp[h])
                    proj8(h, 1, wq8, xq8, qpp[h])
                    if p3i < len(p3_units):
                        p3_unit(p3_units[p3i]); p3i += 1
                while p3i < len(p3_units):
                    p3_unit(p3_units[p3i]); p3i += 1

            # ============ attention, per head (software-pipelined) ============
            # Depth-3 pipeline: iteration h emits norm(h-3), sums(h-1)'s
            # add/all-reduce, E(h), recip(h-1), then O(h-2) — each engine's
            # queue only ever holds instructions whose operands are already
            # (or imminently) ready, so the cross-engine softmax chain
            # (exp -> mask-mul -> adds -> all-reduce -> recip -> normalize)
            # never stalls the PE or head-of-line-blocks DVE. Softmax sums
            # run OFF the PE: bf16 tree-adds on DVE (2-byte fast mode), a
            # Pool partition_all_reduce (output already broadcast across
            # partitions), and a bf16 DVE reciprocal.
            import concourse.bass_isa as bass_isa

            with (
                tc.tile_pool(name="exp", bufs=6) as expp,
                tc.tile_pool(name="emp", bufs=16) as emp,
                tc.tile_pool(name="sump", bufs=3) as sump,
                tc.tile_pool(name="repp", bufs=4) as repp,
                tc.tile_pool(name="opop", bufs=4) as opop,
                tc.tile_pool(name="pse", bufs=2, space="PSUM") as pse,
                tc.tile_pool(name="pso", bufs=2, space="PSUM") as pso,  # 2x2+2x2 = 8
            ):

                def emit_e(h):
                    # E psum lands in 2-bank [128, 1024] pairs so one ACT
                    # exp covers two k-chunks (halves ACT instruction count)
                    em = []
                    for p in range(2):
                        pe2 = pse.tile([128, 2 * N], F32, name="pe", tag="pe")
                        for half in range(2):
                            kc = 2 * p + half
                            for mp in range(2):
                                nc.tensor.matmul(
                                    pe2[:, half * N : (half + 1) * N],
                                    kpp[h][:, mp, kc * 128 : (kc + 1) * 128],
                                    qpp[h][:, mp, :],
                                    start=(mp == 0),
                                    stop=(mp == 1),
                                )
                        ex2 = expp.tile([128, 2, N], BF, name="ex", tag="ex")
                        nc.scalar.activation(
                            ex2.rearrange("p t n -> p (t n)"),
                            pe2,
                            AF.Exp,
                            scale=ESCALE,
                        )
                        for half in range(2):
                            emt = emp.tile([128, N], BF, name="em", tag="em")
                            nc.vector.tensor_mul(
                                emt, ex2[:, half, :], mask2[:, p, half, :]
                            )
                            em.append(emt)
                    return em

                def emit_sums(h, em):
                    """bf16 tree-adds + Pool all-reduce (recip emitted later)"""
                    t0 = sump.tile([128, N], BF, name="t0", tag="t0")
                    nc.vector.tensor_add(t0, em[0], em[1])
                    t1 = sump.tile([128, N], BF, name="t1", tag="t1")
                    nc.vector.tensor_add(t1, em[2], em[3])
                    s_all = sump.tile([128, N], BF, name="s_all", tag="t0")
                    nc.vector.tensor_add(s_all, t0, t1)
                    s_red = sump.tile([128, N], BF, name="s_red", tag="t1")
                    nc.gpsimd.partition_all_reduce(
                        s_red, s_all, channels=128, reduce_op=bass_isa.ReduceOp.add
                    )
                    return s_red

                def emit_recip(s_red):
                    rep = repp.tile([128, N], BF, name="rep", tag="rep")
                    with nc.allow_low_precision(reason="softmax 1/sum"):
                        nc.vector.reciprocal(rep, s_red)
                    return rep

                def emit_o(h, em):
                    """O matmuls into a 2-bank psum pair; one ACT evac"""
                    po2 = pso.tile([128, 2 * N], F32, name="po", tag="po")
                    for mp in range(2):
                        for kc in range(KC):
                            nc.tensor.matmul(
                                po2[:, mp * N : (mp + 1) * N],
                                vp2[kc][:, h * 256 + mp * 128 : h * 256 + (mp + 1) * 128],
                                em[kc],
                                start=(kc == 0),
                                stop=(kc == KC - 1),
                            )
                    opo = opop.tile([128, 2, N], BF, name="opo", tag="opo")
                    nc.scalar.copy(
                        opo.rearrange("p t n -> p (t n)"), po2
                    )
                    return opo

                def emit_norm(h, opo, rep):
                    """normalize O into OT tiles (bf16 fast muls, DVE/Pool;
                    late heads all-DVE so the P7 warm-up isn't gated on Pool)"""
                    hp, hs = h // 2, h % 2
                    for mp in range(2):
                        for s in range(2):
                            eng = nc.gpsimd if (mp == 1 and h < 14) else nc.vector
                            eng.tensor_mul(
                                ot[hp][hs * 64 : (hs + 1) * 64, 2 * mp + s, :],
                                opo[s * 64 : (s + 1) * 64, mp, :],
                                rep[s * 64 : (s + 1) * 64, :],
                            )

                em_q, sr_q, rep_q, opo_q = {}, {}, {}, {}
                for h in range(H):
                    if h >= 3:
                        emit_norm(h - 3, opo_q.pop(h - 3), rep_q[h - 3])
                    if h >= 1:
                        sr_q[h - 1] = emit_sums(h - 1, em_q[h - 1])
                    if h >= 14:
                        # last iterations: O before E so the opo evacs reach
                        # ACT ahead of the exps and free the psum pairs the
                        # epilogue's O(14)/O(15) need
                        opo_q[h - 2] = emit_o(h - 2, em_q.pop(h - 2))
                    em_q[h] = emit_e(h)
                    if h >= 1:
                        rep_q[h - 1] = emit_recip(sr_q.pop(h - 1))
                    if 2 <= h < 14:
                        opo_q[h - 2] = emit_o(h - 2, em_q.pop(h - 2))
                # epilogue: h-1=15 sums/recip, O(14), O(15), norms 13..15
                sr_q[15] = emit_sums(15, em_q[15])
                rep_q[15] = emit_recip(sr_q.pop(15))
                opo_q[14] = emit_o(14, em_q.pop(14))
                emit_norm(13, opo_q.pop(13), rep_q[13])
                opo_q[15] = emit_o(15, em_q.pop(15))
                emit_norm(14, opo_q.pop(14), rep_q[14])
                emit_norm(15, opo_q.pop(15), rep_q[15])

            # ============ P7: output projection (bf16) ============
            # The first four psum groups run contraction chunks hp=0..5
            # before any hp>=6 step, so the PE keeps streaming while the
            # last two heads\' normalized OT tiles are still being written.
            with (
                tc.tile_pool(name="outst", bufs=4) as outstp,
                tc.tile_pool(name="psf", bufs=4, space="PSUM") as psf,
            ):
                units = [(mi, ncc) for mi in range(M) for ncc in range(NC)]

                def p7_mms(pf, mi, ncc, dvh, hps, first, last):
                    for hp in hps:
                        nc.tensor.matmul(
                            pf,
                            ot[hp][:, mi, ncc * 128 : (ncc + 1) * 128],
                            wob[:, hp, dvh * 512 : (dvh + 1) * 512],
                            start=(hp == first),
                            stop=(hp == last),
                        )

                def p7_finish(u, pf0, pf1):
                    mi, ncc = u
                    ost = outstp.tile([128, 2, 512], F32, name="ost", tag="ost")
                    nc.scalar.copy(ost[:, 0, :], pf0)
                    nc.vector.tensor_copy(ost[:, 1, :], pf1)
                    nc.sync.dma_start(
                        out_d.ap()[ncc * 128 : (ncc + 1) * 128, mi, :],
                        ost.rearrange("p a b -> p (a b)"),
                    )

                # warm stretch: 4 psum groups of hp0..5 for the first 2 units
                warm = []
                for u in units[:4]:
                    mi, ncc = u
                    pfs = []
                    for dvh in range(2):
                        pf = psf.tile([128, 512], F32, name="pf", tag="pf")
                        p7_mms(pf, mi, ncc, dvh, range(6), 0, HP - 1)
                        pfs.append(pf)
                    warm.append((u, pfs))
                for u, pfs in warm:
                    mi, ncc = u
                    for dvh in range(2):
                        p7_mms(pfs[dvh], mi, ncc, dvh, range(6, HP), 0, HP - 1)
                    p7_finish(u, *pfs)
                for u in units[4:-1]:
                    mi, ncc = u
                    pfs = []
                    for dvh in range(2):
                        pf = psf.tile([128, 512], F32, name="pf", tag="pf")
                        p7_mms(pf, mi, ncc, dvh, range(HP), 0, HP - 1)
                        pfs.append(pf)
                    p7_finish(u, *pfs)
                # last unit: quarter-column evac/DMA pipeline for a
                # short end-of-kernel drain
                mi, ncc = units[-1]
                for dvh in range(2):
                    pf = psf.tile([128, 512], F32, name="pf", tag="pf")
                    p7_mms(pf, mi, ncc, dvh, range(HP), 0, HP - 1)
                    osh = outstp.tile([128, 512], F32, name="osh", tag="osh")
                    eng = nc.scalar if dvh == 0 else nc.vector
                    ecopy(eng, osh, pf)
                    nc.sync.dma_start(
                        out_d.ap()[
                            ncc * 128 : (ncc + 1) * 128,
                            mi,
                            dvh * 512 : (dvh + 1) * 512,
                        ],
                        osh,
                    )

    nc.compile()
    return nc


_NC_CACHE = None


def _get_nc():
    global _NC_CACHE
    if _NC_CACHE is None:
        _NC_CACHE = build_nc()
    return _NC_CACHE


def _dr_pack(a):
    """[512, ...] rows dq = c*256 + i*128 + p -> [128 = p, 2 = c, 2 = i, ...]"""
    s = a.shape[1:]
    return a.reshape(2, 2, 128, *s).transpose(2, 0, 1, *range(3, 3 + len(s)))


def kernel(Q, K, mask, w_q, w_k, w_v, w_o):
    from concourse.bass_utils import run_bass_kernel_spmd

    Q = np.asarray(Q, dtype=np.float32)
    K = np.asarray(K, dtype=np.float32)
    mask = np.asarray(mask)
    w_q = np.asarray(w_q, dtype=np.float32)
    w_k = np.asarray(w_k, dtype=np.float32)
    w_v = np.asarray(w_v, dtype=np.float32)
    w_o = np.asarray(w_o, dtype=np.float32)

    wq8 = np.ascontiguousarray(_dr_pack(w_q * WS)).astype(NPF8)
    wk8 = np.ascontiguousarray(_dr_pack(w_k * WS)).astype(NPF8)
    wvb = np.ascontiguousarray(
        w_v.reshape(4, 128, DV).transpose(1, 0, 2)
    ).astype(NPBF)
    wob = np.ascontiguousarray(
        w_o.reshape(HP, 128, DV).transpose(1, 0, 2)
    ).astype(NPBF)

    in_maps = []
    for b in range(B):
        qt = np.ascontiguousarray(Q[b].transpose(2, 1, 0))   # [DQ, M, N]
        kt = np.ascontiguousarray(K[b].transpose(2, 1, 0))   # [DK, M, KN]
        in_maps.append(
            {
                "QT8": np.ascontiguousarray(_dr_pack(qt)).astype(NPF8),
                "KT8": np.ascontiguousarray(_dr_pack(kt)).astype(NPF8),
                "KTB": np.ascontiguousarray(
                    kt.reshape(4, 128, M, KN).transpose(1, 0, 2, 3)
                ).astype(NPBF),
                "MT": np.ascontiguousarray(
                    mask[b].T.reshape(2, 2, 128, N).transpose(2, 0, 1, 3)
                ).astype(NPBF),
                "WQ8": wq8,
                "WK8": wk8,
                "WVB": wvb,
                "WOB": wob,
            }
        )

    nc = _get_nc()
    r = run_bass_kernel_spmd(nc, in_maps, core_ids=list(range(B)), trace=False)
    return np.stack([r.results[b]["out"] for b in range(B)], axis=0)


if __name__ == "__main__":
    rng = np.random.default_rng(0)
    inputs = {
        "Q": rng.standard_normal((B, N, M, DQ), dtype=np.float32),
        "K": rng.standard_normal((B, KN, M, DK), dtype=np.float32),
        "mask": rng.integers(0, 2, (B, N, KN)).astype(np.int32),
        "w_q": (rng.standard_normal((DQ, DV), dtype=np.float32) * 0.02),
        "w_k": (rng.standard_normal((DK, DV), dtype=np.float32) * 0.02),
        "w_v": (rng.standard_normal((DK, DV), dtype=np.float32) * 0.02),
        "w_o": (rng.standard_normal((DV, DV), dtype=np.float32) * 0.02),
    }
    out = kernel(**inputs)
    print("out", out.shape, out.dtype, float(np.abs(out).max()))



# revision 27
# speedup vs baseline: 1.1154x; 1.1154x over previous
"""EquiMHA Trainium2 kernel.

Data-parallel over batch B=8 across the 8 NeuronCores (one batch element per
core, weights replicated, no collectives).

Per-core computation for batch b (N=512, M=4, KN=512, DQ=DK=512, DV=1024,
H=16, D=64):
  Qp = Q[b] @ w_q, Kp = K[b] @ w_k, Vp = K[b] @ w_v
  E[h,n,k] = sum_{m,d} Qp[n,m,h*64+d] Kp[k,m,h*64+d] / 32
  A = masked_softmax(E)        (max-subtraction skipped: |E|/32 <= ~2, and the
                                max cancels exactly up to the +eps term)
  O[n,m,h*64+d] = sum_k A[h,n,k] Vp[k,m,h*64+d]
  out = O @ w_o

Precision strategy (tolerance is 2e-2; measured pipeline error ~9e-3):
  - Q/K-side projections run in fp8e4m3 with DoubleRow perf mode (2x PE
    rate, 256-deep contraction per pass); stationary tiles are 128 columns
    wide (a head PAIR), which halves the projection matmul count vs 64-col.
  - The projected qpp/kpp head tiles are themselves stored in fp8e4m3, so
    the E (scores) matmuls also run DoubleRow at 2x rate with the full
    256-deep (m,d) contraction in a single pass: 1 matmul per 128-k chunk.
  - The mask is folded into the E PSUM on the PE: one extra DoubleRow
    matmul per k-chunk with an fp8e5m2 identity stationary (value 1024) and
    an fp8e5m2 mask-bias moving operand (-4096 where masked) adds -2^22 to
    masked logits; after the exp scale that's -32, so exp()==~2e-14==0.
    This removes all 64 DVE mask-multiplies from the softmax chain.
  - V path (P3), A@V (O), and the output projection (P7) stay bf16: fp8
    anywhere on the V/O path adds ~3-4e-2 relative error (V-side errors are
    not softmax-damped), which blows the 2e-2 budget.

Schedule strategy (the cost model charges matmuls by output free size and
models the PE p-state ramp, so the kernel keeps the PE busy end to end):
  - ~44 zero warm-up matmuls fill the initial input-DMA window so the PE
    p-state is fully ramped when the first projection lands.
  - Input DMAs are sliced (per m chunk) and ordered to match first use.
  - P3 (Vp, bf16) is split: 16 units woven through the P1 era, 16 through
    the attention head loop, so the PE never idles while the ACT/DVE
    engines drain the PSUM-evacuation backlog.
  - Attention head loop is software-pipelined with a static engine
    assignment (Pool cannot touch PSUM): exp on ACT; partial-sum adds,
    reciprocal on DVE; partition_all_reduce plus 2 of 4 norm muls on Pool,
    the other 2 on DVE; O-psum evacuation alternates ACT/DVE by head.
"""

import numpy as np
import ml_dtypes

import concourse.bacc as bacc
import concourse.mybir as mybir
import concourse.tile as tile

F32 = mybir.dt.float32
F8 = mybir.dt.float8e4
F8E5 = mybir.dt.float8e5
BF = mybir.dt.bfloat16
AF = mybir.ActivationFunctionType
DR = mybir.MatmulPerfMode.DoubleRow

NPF8 = ml_dtypes.float8_e4m3
NPF8E5 = ml_dtypes.float8_e5m2
NPBF = ml_dtypes.bfloat16

B, N, M, KN = 8, 512, 4, 512
DQ, DK, DV, H = 512, 512, 1024, 16
D = DV // H
HP = H // 2          # head pairs (P7 contraction chunks)
KC = KN // 128       # k chunks
NC = N // 128        # n chunks
WS = 64.0            # host pre-scale for fp8 weights
SCALE = 1.0 / 32.0   # 1/sqrt(DV)
ESCALE = SCALE / (WS * WS)  # fused into exp
MBIAS = -4096.0      # fp8e5 mask bias; with ID 1024 adds -32 to the logits
NWARM = 44           # p-state warm-up matmuls


def build_nc():
    nc = bacc.Bacc("TRN2", target_bir_lowering=False, debug=False, num_devices=8)

    qt8_d = nc.dram_tensor("QT8", [128, 2, 2, M, N], F8, kind="ExternalInput")
    kt8_d = nc.dram_tensor("KT8", [128, 2, 2, M, KN], F8, kind="ExternalInput")
    ktb_d = nc.dram_tensor("KTB", [128, 4, M, KN], BF, kind="ExternalInput")
    mb8_d = nc.dram_tensor("MB8", [128, 2, KC, N], F8E5, kind="ExternalInput")
    id8_d = nc.dram_tensor("ID8", [128, 2, 128], F8E5, kind="ExternalInput")
    wq8_d = nc.dram_tensor("WQ8", [128, 2, 2, DV], F8, kind="ExternalInput")
    wk8_d = nc.dram_tensor("WK8", [128, 2, 2, DV], F8, kind="ExternalInput")
    wvb_d = nc.dram_tensor("WVB", [128, 4, DV], BF, kind="ExternalInput")
    wob_d = nc.dram_tensor("WOB", [128, 8, DV], BF, kind="ExternalInput")
    out_d = nc.dram_tensor("out", [N, M, DV], F32, kind="ExternalOutput")

    import concourse.bass_isa as bass_isa

    # --- greedy projected-load balancer for evac copies (ns estimates) ---
    eng_ns = {"act": 0.0, "dve": 0.0}

    def pick2(act_cost, dve_cost):
        if eng_ns["act"] + act_cost <= eng_ns["dve"] + dve_cost:
            eng_ns["act"] += act_cost
            return nc.scalar
        eng_ns["dve"] += dve_cost
        return nc.vector

    def ecopy(eng, dst, src):
        if eng is nc.scalar:
            nc.scalar.copy(dst, src)
        else:
            eng.tensor_copy(dst, src)

    with tile.TileContext(nc) as tc:
        with tc.tile_pool(name="persist", bufs=1) as persist:
            vp2 = [persist.tile([128, M * DV], BF, name=f"vp2_{kc}") for kc in range(KC)]
            # qq/kk [(h2,d) 128, s 2, mp 2, n] fp8: PSUM-native layout so each
            # projection evacuation is one full-128-partition copy; E slices
            # the h2 half and accumulates over s in two 64-partition DR passes
            qq = [persist.tile([128, 2, 2, N], F8, name=f"qq{g}") for g in range(HP)]
            kk = [persist.tile([128, 2, 2, KN], F8, name=f"kk{g}") for g in range(HP)]
            ot = [persist.tile([128, M, N], BF, name=f"ot{hp}") for hp in range(HP)]
            wob = persist.tile([128, HP, DV], BF, name="wob")
            mb8 = persist.tile([128, 2, KC, N], F8E5, name="mb8")
            id8 = persist.tile([128, 2, 128], F8E5, name="id8")
            xkb = persist.tile([128, 4, M, KN], BF, name="xkb")
            wvb = persist.tile([128, 4, DV], BF, name="wvb")

            # ---- p-state warm-up: keep the PE busy during the input DMAs ----
            with (
                tc.tile_pool(name="warm", bufs=1) as warmp,
                tc.tile_pool(name="warmps", bufs=1, space="PSUM") as warmpsp,
            ):
                wz = warmp.tile([128, 128], BF, name="wz")
                nc.vector.memzero(wz)
                wps = warmpsp.tile([128, 128], F32, name="wps")
                for _ in range(NWARM):
                    nc.tensor.matmul(wps, wz, wz, start=True, stop=True)

            # ============ P1/P2 projections (fp8 DR, head-pair stationary) ====
            with (
                tc.tile_pool(name="xk8", bufs=1) as xk8p,
                tc.tile_pool(name="w8", bufs=1) as w8p,
                tc.tile_pool(name="ppj", bufs=3, space="PSUM") as ppj,
                tc.tile_pool(name="ppv", bufs=2, space="PSUM") as ppv1,
            ):
                xk8 = xk8p.tile([128, 2, 2, M, KN], F8, name="xk8")
                wk8 = w8p.tile([128, 2, 2, DV], F8, name="wk8")
                xq8 = xk8p.tile([128, 2, 2, M, N], F8, name="xq8")
                wq8 = w8p.tile([128, 2, 2, DV], F8, name="wq8")
                # DMA order tracks PE consumption order (the model's DMA
                # engines are a single serial resource)
                for half in range(2):
                    nc.sync.dma_start(
                        wk8[:, :, :, half * 512 : (half + 1) * 512],
                        wk8_d.ap()[:, :, :, half * 512 : (half + 1) * 512],
                    )
                for m in (0, 2, 1, 3):
                    nc.sync.dma_start(xk8[:, :, :, m, :], kt8_d.ap()[:, :, :, m, :])
                for c in range(4):
                    nc.sync.dma_start(wvb[:, c, :], wvb_d.ap()[:, c, :])
                    nc.sync.dma_start(xkb[:, c, :, :], ktb_d.ap()[:, c, :, :])
                nc.sync.dma_start(wq8, wq8_d.ap())
                for m in (0, 2, 1, 3):
                    nc.sync.dma_start(xq8[:, :, :, m, :], qt8_d.ap()[:, :, :, m, :])
                nc.sync.dma_start(id8, id8_d.ap())
                nc.sync.dma_start(mb8, mb8_d.ap())
                nc.sync.dma_start(wob, wob_d.ap())

                def proj_pair(hp8, s, w_sb, x_sb, dst):
                    """One head-pair, one m-parity: psum [128=(h2,d), mp, n];
                    single full-partition evacuation into dst[:, s, :, :]"""
                    pq = ppj.tile([128, 2, N], F32, name="pq", tag="pq")
                    for mp in range(2):
                        m = 2 * mp + s
                        for c in range(2):
                            nc.tensor.matmul(
                                pq[:, mp, :],
                                w_sb[:, c, :, hp8 * 128 : (hp8 + 1) * 128],
                                x_sb[:, c, :, m, :],
                                start=(c == 0),
                                stop=(c == 1),
                                perf_mode=DR,
                            )
                    ecopy(pick2(1038, 1192), dst[:, s, :, :], pq)

                def p3_unit(u, pool, eng=None):
                    mi, kc, dvh = u
                    off = mi * 64
                    pv = pool.tile([128, 512], F32, name="pv", tag="pv")
                    for c in range(4):
                        nc.tensor.matmul(
                            pv,
                            xkb[:, c, mi, kc * 128 : (kc + 1) * 128],
                            wvb[:, c, dvh * 512 : (dvh + 1) * 512],
                            start=(c == 0),
                            stop=(c == 3),
                        )
                    v4 = vp2[kc].rearrange("p (h c) -> p h c", h=H)
                    ecopy(
                        eng if eng is not None else pick2(612, 658),
                        v4[:, dvh * 8 : (dvh + 1) * 8, off : off + 64],
                        pv.rearrange("p (h d) -> p h d", h=8),
                    )

                p3_units = [
                    (mi, kc, dvh)
                    for dvh in range(2)
                    for kc in range(KC)
                    for mi in range(M)
                ]

                for s in range(2):
                    for hp8 in range(HP):
                        proj_pair(hp8, s, wk8, xk8, kk[hp8])
                # P1 with the 16 dvh=0 P3 units woven in
                for hp8 in range(HP):
                    proj_pair(hp8, 0, wq8, xq8, qq[hp8])
                    p3_unit(p3_units[2 * hp8], ppv1)
                    proj_pair(hp8, 1, wq8, xq8, qq[hp8])
                    p3_unit(p3_units[2 * hp8 + 1], ppv1)

            # ============ attention (software-pipelined head loop) ============
            with (
                tc.tile_pool(name="exp", bufs=8) as expp,
                tc.tile_pool(name="sump", bufs=3) as sump,
                tc.tile_pool(name="repp", bufs=5) as repp,
                tc.tile_pool(name="opop", bufs=4) as opop,
                tc.tile_pool(name="pse", bufs=2, space="PSUM") as pse,
                tc.tile_pool(name="pso", bufs=1, space="PSUM") as pso,
                tc.tile_pool(name="ppv2", bufs=2, space="PSUM") as ppv2,
            ):

                def emit_e(h):
                    """E DR matmuls (two 64-partition s-passes) + mask-bias
                    matmul + exp -> masked em"""
                    g, h2 = h // 2, h % 2
                    sl = slice(h2 * 64, (h2 + 1) * 64)
                    em = []
                    for p in range(2):
                        pe2 = pse.tile([128, 2, N], F32, name="pe", tag="pe")
                        for half in range(2):
                            kc = 2 * p + half
                            for s in range(2):
                                nc.tensor.matmul(
                                    pe2[:, half, :],
                                    kk[g][sl, s, :, kc * 128 : (kc + 1) * 128],
                                    qq[g][sl, s, :, :],
                                    start=(s == 0),
                                    stop=False,
                                    perf_mode=DR,
                                )
                            nc.tensor.matmul(
                                pe2[:, half, :],
                                id8,
                                mb8[:, :, kc, :],
                                start=False,
                                stop=True,
                                perf_mode=DR,
                            )
                        ex2 = expp.tile([128, 2, N], BF, name="ex", tag="ex")
                        nc.scalar.activation(
                            ex2.rearrange("p t n -> p (t n)"),
                            pe2.rearrange("p t n -> p (t n)"),
                            AF.Exp,
                            scale=ESCALE,
                        )
                        em.append(ex2)
                    return em

                def emit_sums(h, em):
                    """bf16 tree-adds (DVE) + Pool all-reduce"""
                    t01 = sump.tile([128, 2, N], BF, name="t01", tag="t01")
                    nc.vector.tensor_add(t01, em[0], em[1])
                    s_all = sump.tile([128, N], BF, name="s_all", tag="s_all")
                    nc.vector.tensor_add(s_all, t01[:, 0, :], t01[:, 1, :])
                    s_red = sump.tile([128, N], BF, name="s_red", tag="s_red")
                    nc.gpsimd.partition_all_reduce(
                        s_red, s_all, channels=128, reduce_op=bass_isa.ReduceOp.add
                    )
                    return s_red

                def emit_recip(s_red):
                    rep = repp.tile([128, N], BF, name="rep", tag="rep")
                    with nc.allow_low_precision(reason="softmax 1/sum"):
                        nc.vector.reciprocal(rep, s_red)
                    return rep

                def emit_o(h, em):
                    """O matmuls into a 2-bank psum; evac alternates ACT/DVE"""
                    po2 = pso.tile([128, 2, N], F32, name="po", tag="po")
                    for mp in range(2):
                        for kc in range(KC):
                            nc.tensor.matmul(
                                po2[:, mp, :],
                                vp2[kc][:, h * 256 + mp * 128 : h * 256 + (mp + 1) * 128],
                                em[kc // 2][:, kc % 2, :],
                                start=(kc == 0),
                                stop=(kc == KC - 1),
                            )
                    opo = opop.tile([128, 2, N], BF, name="opo", tag="opo")
                    eng = nc.scalar if h % 2 == 0 else nc.vector
                    ecopy(eng, opo.rearrange("p t n -> p (t n)"),
                          po2.rearrange("p t n -> p (t n)"))
                    return opo

                def emit_norm(h, opo, rep, dve_all=False):
                    """normalize O into OT tiles: 2 muls on DVE, 2 on Pool"""
                    hp, hs = h // 2, h % 2
                    for i, (mp, s) in enumerate(
                        ((0, 0), (0, 1), (1, 0), (1, 1))
                    ):
                        eng = nc.vector if (dve_all or i % 2 == 0) else nc.gpsimd
                        eng.tensor_mul(
                            ot[hp][hs * 64 : (hs + 1) * 64, 2 * mp + s, :],
                            opo[s * 64 : (s + 1) * 64, mp, :],
                            rep[s * 64 : (s + 1) * 64, :],
                        )

                em_q, rep_q, opo_q = {}, {}, {}
                # dvh=1 P3 units woven 2/head through heads 0..7 so all of
                # them are emitted before O(8) consumes their vp2 columns.
                # O lags E by 3 heads so the last O-chains (and their evac/
                # norm engine work) ride the P7 warm stretch.
                p3i = 16
                for h in range(H):
                    if h >= 3:
                        opo_q[h - 3] = emit_o(h - 3, em_q.pop(h - 3))
                    em_q[h] = emit_e(h)
                    while p3i < min(32, 16 + 2 * (h + 1)):
                        p3_unit(p3_units[p3i], ppv2,
                                eng=nc.scalar if p3i % 2 == 0 else nc.vector)
                        p3i += 1
                    if h >= 1:
                        rep_q[h - 1] = emit_recip(emit_sums(h - 1, em_q[h - 1]))
                    if h >= 4:
                        emit_norm(h - 4, opo_q.pop(h - 4), rep_q.pop(h - 4))
                # epilogue: O(13..15) + norms 12..15 overlap the P7 warm units
                rep_q[15] = emit_recip(emit_sums(15, em_q[15]))
                opo_q[13] = emit_o(13, em_q.pop(13))
                emit_norm(12, opo_q.pop(12), rep_q.pop(12))
                opo_q[14] = emit_o(14, em_q.pop(14))
                emit_norm(13, opo_q.pop(13), rep_q.pop(13))
                opo_q[15] = emit_o(15, em_q.pop(15))
                emit_norm(14, opo_q.pop(14), rep_q.pop(14))
                emit_norm(15, opo_q.pop(15), rep_q.pop(15), dve_all=True)

            # ============ P7: output projection (bf16) ============
            # The first four psum groups run contraction chunks hp=0..5
            # before any hp>=6 step, so the PE keeps streaming while the
            # last two heads' normalized OT tiles are still being written.
            with (
                tc.tile_pool(name="outst", bufs=4) as outstp,
                tc.tile_pool(name="psf", bufs=4, space="PSUM") as psf,
            ):
                units = [(mi, ncc) for mi in range(M) for ncc in range(NC)]

                def p7_mms(pf, mi, ncc, dvh, hps, first, last):
                    for hp in hps:
                        nc.tensor.matmul(
                            pf,
                            ot[hp][:, mi, ncc * 128 : (ncc + 1) * 128],
                            wob[:, hp, dvh * 512 : (dvh + 1) * 512],
                            start=(hp == first),
                            stop=(hp == last),
                        )

                def p7_finish(u, pf0, pf1):
                    mi, ncc = u
                    ost = outstp.tile([128, 2, 512], F32, name="ost", tag="ost")
                    ecopy(pick2(612, 658), ost[:, 0, :], pf0)
                    ecopy(pick2(612, 658), ost[:, 1, :], pf1)
                    nc.sync.dma_start(
                        out_d.ap()[ncc * 128 : (ncc + 1) * 128, mi, :],
                        ost.rearrange("p a b -> p (a b)"),
                    )

                # warm stretch: 4 psum groups of hp0..5 for the first 2 units
                warm = []
                for u in units[:4]:
                    mi, ncc = u
                    pfs = []
                    for dvh in range(2):
                        pf = psf.tile([128, 512], F32, name="pf", tag="pf")
                        p7_mms(pf, mi, ncc, dvh, range(6), 0, HP - 1)
                        pfs.append(pf)
                    warm.append((u, pfs))
                for u, pfs in warm:
                    mi, ncc = u
                    for dvh in range(2):
                        p7_mms(pfs[dvh], mi, ncc, dvh, range(6, HP), 0, HP - 1)
                    p7_finish(u, *pfs)
                for u in units[4:-2]:
                    mi, ncc = u
                    pfs = []
                    for dvh in range(2):
                        pf = psf.tile([128, 512], F32, name="pf", tag="pf")
                        p7_mms(pf, mi, ncc, dvh, range(HP), 0, HP - 1)
                        pfs.append(pf)
                    p7_finish(u, *pfs)
                # last two units: per-half evac/DMA pipeline so the final
                # evacuation and store overlap the preceding unit's matmuls
                for u in units[-2:]:
                    mi, ncc = u
                    for dvh in range(2):
                        pf = psf.tile([128, 512], F32, name="pf", tag="pf")
                        p7_mms(pf, mi, ncc, dvh, range(HP), 0, HP - 1)
                        osh = outstp.tile([128, 512], F32, name="osh", tag="osh")
                        eng = nc.scalar if dvh == 0 else nc.vector
                        ecopy(eng, osh, pf)
                        nc.sync.dma_start(
                            out_d.ap()[
                                ncc * 128 : (ncc + 1) * 128,
                                mi,
                                dvh * 512 : (dvh + 1) * 512,
                            ],
                            osh,
                        )

    nc.compile()
    return nc


_NC_CACHE = None


def _get_nc():
    global _NC_CACHE
    if _NC_CACHE is None:
        _NC_CACHE = build_nc()
    return _NC_CACHE


def _dr_pack(a):
    """[512, ...] rows dq = c*256 + i*128 + p -> [128 = p, 2 = c, 2 = i, ...]"""
    s = a.shape[1:]
    return a.reshape(2, 2, 128, *s).transpose(2, 0, 1, *range(3, 3 + len(s)))


def kernel(Q, K, mask, w_q, w_k, w_v, w_o):
    from concourse.bass_utils import run_bass_kernel_spmd

    Q = np.asarray(Q, dtype=np.float32)
    K = np.asarray(K, dtype=np.float32)
    mask = np.asarray(mask)
    w_q = np.asarray(w_q, dtype=np.float32)
    w_k = np.asarray(w_k, dtype=np.float32)
    w_v = np.asarray(w_v, dtype=np.float32)
    w_o = np.asarray(w_o, dtype=np.float32)

    wq8 = np.ascontiguousarray(_dr_pack(w_q * WS)).astype(NPF8)
    wk8 = np.ascontiguousarray(_dr_pack(w_k * WS)).astype(NPF8)
    wvb = np.ascontiguousarray(
        w_v.reshape(4, 128, DV).transpose(1, 0, 2)
    ).astype(NPBF)
    wob = np.ascontiguousarray(
        w_o.reshape(HP, 128, DV).transpose(1, 0, 2)
    ).astype(NPBF)
    id8 = np.zeros((128, 2, 128), NPF8E5)
    id8[:, 0, :] = (np.eye(128) * 1024.0).astype(NPF8E5)

    in_maps = []
    for b in range(B):
        qt = np.ascontiguousarray(Q[b].transpose(2, 1, 0))   # [DQ, M, N]
        kt = np.ascontiguousarray(K[b].transpose(2, 1, 0))   # [DK, M, KN]
        mb = np.zeros((128, 2, KC, N), NPF8E5)
        # mask[b] is [N, KN]; mb[p, 0, kc, n] = MBIAS where mask[n, kc*128+p]==0
        mt = mask[b].T.reshape(KC, 128, N).transpose(1, 0, 2)  # [p, kc, n]
        mb[:, 0, :, :] = (MBIAS * (1 - mt)).astype(NPF8E5)
        in_maps.append(
            {
                "QT8": np.ascontiguousarray(_dr_pack(qt)).astype(NPF8),
                "KT8": np.ascontiguousarray(_dr_pack(kt)).astype(NPF8),
                "KTB": np.ascontiguousarray(
                    kt.reshape(4, 128, M, KN).transpose(1, 0, 2, 3)
                ).astype(NPBF),
                "MB8": mb,
                "ID8": id8,
                "WQ8": wq8,
                "WK8": wk8,
                "WVB": wvb,
                "WOB": wob,
            }
        )

    nc = _get_nc()
    r = run_bass_kernel_spmd(nc, in_maps, core_ids=list(range(B)), trace=False)
    return np.stack([r.results[b]["out"] for b in range(B)], axis=0)


if __name__ == "__main__":
    rng = np.random.default_rng(0)
    inputs = {
        "Q": rng.standard_normal((B, N, M, DQ), dtype=np.float32),
        "K": rng.standard_normal((B, KN, M, DK), dtype=np.float32),
        "mask": rng.integers(0, 2, (B, N, KN)).astype(np.int32),
        "w_q": (rng.standard_normal((DQ, DV), dtype=np.float32) * 0.02),
        "w_k": (rng.standard_normal((DK, DV), dtype=np.float32) * 0.02),
        "w_v": (rng.standard_normal((DK, DV), dtype=np.float32) * 0.02),
        "w_o": (rng.standard_normal((DV, DV), dtype=np.float32) * 0.02),
    }
    out = kernel(**inputs)
    print("out", out.shape, out.dtype, float(np.abs(out).max()))


# revision 43
# speedup vs baseline: 1.1323x; 1.0151x over previous
"""EquiMHA Trainium2 kernel.

Data-parallel over batch B=8 across the 8 NeuronCores (one batch element per
core, weights replicated, no collectives).

Per-core computation for batch b (N=512, M=4, KN=512, DQ=DK=512, DV=1024,
H=16, D=64):
  Qp = Q[b] @ w_q, Kp = K[b] @ w_k, Vp = K[b] @ w_v
  E[h,n,k] = sum_{m,d} Qp[n,m,h*64+d] Kp[k,m,h*64+d] / 32
  A = masked_softmax(E)        (max-subtraction skipped: |E|/32 <= ~2, and the
                                max cancels exactly up to the +eps term)
  O[n,m,h*64+d] = sum_k A[h,n,k] Vp[k,m,h*64+d]
  out = O @ w_o

Precision strategy (tolerance is 2e-2; measured pipeline error ~9e-3):
  - Q/K-side projections run in fp8e4m3 with DoubleRow perf mode (2x PE
    rate, 256-deep contraction per pass); stationary tiles are 128 columns
    wide (a head PAIR), which halves the projection matmul count vs 64-col.
  - The projected qpp/kpp head tiles are themselves stored in fp8e4m3, so
    the E (scores) matmuls also run DoubleRow at 2x rate with the full
    256-deep (m,d) contraction in a single pass: 1 matmul per 128-k chunk.
  - The mask is folded into the E PSUM on the PE: one extra DoubleRow
    matmul per k-chunk with an fp8e5m2 identity stationary (value 1024) and
    an fp8e5m2 mask-bias moving operand (-4096 where masked) adds -2^22 to
    masked logits; after the exp scale that's -32, so exp()==~2e-14==0.
    This removes all 64 DVE mask-multiplies from the softmax chain.
  - V path (P3), A@V (O), and the output projection (P7) stay bf16: fp8
    anywhere on the V/O path adds ~3-4e-2 relative error (V-side errors are
    not softmax-damped), which blows the 2e-2 budget.

Schedule strategy (the cost model charges matmuls by output free size and
models the PE p-state ramp, so the kernel keeps the PE busy end to end):
  - ~44 zero warm-up matmuls fill the initial input-DMA window so the PE
    p-state is fully ramped when the first projection lands.
  - Input DMAs are sliced (per m chunk) and ordered to match first use.
  - P3 (Vp, bf16) is split: 16 units woven through the P1 era, 16 through
    the attention head loop, so the PE never idles while the ACT/DVE
    engines drain the PSUM-evacuation backlog.
  - Attention head loop is software-pipelined with a static engine
    assignment (Pool cannot touch PSUM): exp on ACT; partial-sum adds,
    reciprocal on DVE; partition_all_reduce plus 2 of 4 norm muls on Pool,
    the other 2 on DVE; O-psum evacuation alternates ACT/DVE by head.
"""

import numpy as np
import ml_dtypes

import concourse.bacc as bacc
import concourse.mybir as mybir
import concourse.tile as tile

F32 = mybir.dt.float32
F8 = mybir.dt.float8e4
F8E5 = mybir.dt.float8e5
BF = mybir.dt.bfloat16
AF = mybir.ActivationFunctionType
DR = mybir.MatmulPerfMode.DoubleRow

NPF8 = ml_dtypes.float8_e4m3
NPF8E5 = ml_dtypes.float8_e5m2
NPBF = ml_dtypes.bfloat16

B, N, M, KN = 8, 512, 4, 512
DQ, DK, DV, H = 512, 512, 1024, 16
D = DV // H
HP = H // 2          # head pairs (P7 contraction chunks)
KC = KN // 128       # k chunks
NC = N // 128        # n chunks
WS = 64.0            # host pre-scale for fp8 weights
SCALE = 1.0 / 32.0   # 1/sqrt(DV)
ESCALE = SCALE / (WS * WS)  # fused into exp
MBIAS = -4096.0      # fp8e5 mask bias; with ID 1024 adds -32 to the logits
NWARM = 44           # p-state warm-up matmuls


def build_nc():
    nc = bacc.Bacc("TRN2", target_bir_lowering=False, debug=False, num_devices=8)

    qt8_d = nc.dram_tensor("QT8", [128, 2, 2, M, N], F8, kind="ExternalInput")
    kt8_d = nc.dram_tensor("KT8", [128, 2, 2, M, KN], F8, kind="ExternalInput")
    ktb_d = nc.dram_tensor("KTB", [128, 4, M, KN], BF, kind="ExternalInput")
    mb8_d = nc.dram_tensor("MB8", [128, 2, KC, N], F8E5, kind="ExternalInput")
    id8_d = nc.dram_tensor("ID8", [128, 2, 128], F8E5, kind="ExternalInput")
    wq8_d = nc.dram_tensor("WQ8", [128, 2, 2, DV], F8, kind="ExternalInput")
    wk8_d = nc.dram_tensor("WK8", [128, 2, 2, DV], F8, kind="ExternalInput")
    wvb_d = nc.dram_tensor("WVB", [128, 4, DV], BF, kind="ExternalInput")
    wob_d = nc.dram_tensor("WOB", [128, 8, DV], BF, kind="ExternalInput")
    out_d = nc.dram_tensor("out", [N, M, DV], F32, kind="ExternalOutput")

    import concourse.bass_isa as bass_isa

    # --- greedy projected-load balancer for evac copies (ns estimates) ---
    eng_ns = {"act": 0.0, "dve": 0.0}

    def pick2(act_cost, dve_cost):
        if eng_ns["act"] + act_cost <= eng_ns["dve"] + dve_cost:
            eng_ns["act"] += act_cost
            return nc.scalar
        eng_ns["dve"] += dve_cost
        return nc.vector

    def ecopy(eng, dst, src):
        if eng is nc.scalar:
            nc.scalar.copy(dst, src)
        else:
            eng.tensor_copy(dst, src)

    with tile.TileContext(nc) as tc:
        with tc.tile_pool(name="persist", bufs=1) as persist:
            vp2 = [persist.tile([128, M * DV], BF, name=f"vp2_{kc}") for kc in range(KC)]
            # qq/kk [(h2,d) 128, s 2, mp 2, n] fp8: PSUM-native layout so each
            # projection evacuation is one full-128-partition copy; E slices
            # the h2 half and accumulates over s in two 64-partition DR passes
            qq = [persist.tile([128, 2, 2, N], F8, name=f"qq{g}") for g in range(HP)]
            kk = [persist.tile([128, 2, 2, KN], F8, name=f"kk{g}") for g in range(HP)]
            ot = [persist.tile([128, M, N], BF, name=f"ot{hp}") for hp in range(HP)]
            wob = persist.tile([128, HP, DV], BF, name="wob")
            mb8 = persist.tile([128, 2, KC, N], F8E5, name="mb8")
            id8 = persist.tile([128, 2, 128], F8E5, name="id8")
            xkb = persist.tile([128, 4, M, KN], BF, name="xkb")
            wvb = persist.tile([128, 4, DV], BF, name="wvb")

            # ---- p-state warm-up: keep the PE busy during the input DMAs ----
            with (
                tc.tile_pool(name="warm", bufs=1) as warmp,
                tc.tile_pool(name="warmps", bufs=1, space="PSUM") as warmpsp,
            ):
                wz = warmp.tile([128, 128], BF, name="wz")
                nc.vector.memzero(wz)
                wps = warmpsp.tile([128, 128], F32, name="wps")
                for _ in range(NWARM):
                    nc.tensor.matmul(wps, wz, wz, start=True, stop=True)

            # ============ P1/P2 projections (fp8 DR, head-pair stationary) ====
            with (
                tc.tile_pool(name="xk8", bufs=1) as xk8p,
                tc.tile_pool(name="w8", bufs=1) as w8p,
            ):
                xk8 = xk8p.tile([128, 2, 2, M, KN], F8, name="xk8")
                wk8 = w8p.tile([128, 2, 2, DV], F8, name="wk8")
                xq8 = xk8p.tile([128, 2, 2, M, N], F8, name="xq8")
                wq8 = w8p.tile([128, 2, 2, DV], F8, name="wq8")
                # DMA order tracks PE consumption order (the model's DMA
                # engines are a single serial resource)
                nc.sync.dma_start(wk8[:, :, :, 0:512], wk8_d.ap()[:, :, :, 0:512])
                for m in (0, 2):
                    nc.sync.dma_start(xk8[:, :, :, m, :], kt8_d.ap()[:, :, :, m, :])
                nc.sync.dma_start(wk8[:, :, :, 512:1024], wk8_d.ap()[:, :, :, 512:1024])
                for m in (1, 3):
                    nc.sync.dma_start(xk8[:, :, :, m, :], kt8_d.ap()[:, :, :, m, :])
                for c in range(4):
                    nc.sync.dma_start(wvb[:, c, :], wvb_d.ap()[:, c, :])
                    nc.sync.dma_start(xkb[:, c, :, :], ktb_d.ap()[:, c, :, :])
                nc.sync.dma_start(wq8, wq8_d.ap())
                for m in (0, 2, 1, 3):
                    nc.sync.dma_start(xq8[:, :, :, m, :], qt8_d.ap()[:, :, :, m, :])
                nc.sync.dma_start(id8, id8_d.ap())
                nc.sync.dma_start(mb8, mb8_d.ap())
                nc.sync.dma_start(wob, wob_d.ap())

                def proj_pair(pool, hp8, s, w_sb, x_sb, dst):
                    """One head-pair, one m-parity: psum [128=(h2,d), mp, n];
                    single full-partition evacuation into dst[:, s, :, :]"""
                    pq = pool.tile([128, 2, N], F32, name="pq", tag="pq")
                    for mp in range(2):
                        m = 2 * mp + s
                        for c in range(2):
                            nc.tensor.matmul(
                                pq[:, mp, :],
                                w_sb[:, c, :, hp8 * 128 : (hp8 + 1) * 128],
                                x_sb[:, c, :, m, :],
                                start=(c == 0),
                                stop=(c == 1),
                                perf_mode=DR,
                            )
                    ecopy(pick2(1038, 1192), dst[:, s, :, :], pq)

                def p3_unit(u, pool, eng=None):
                    mi, kc, dvh = u
                    off = mi * 64
                    pv = pool.tile([128, 512], F32, name="pv", tag="pv")
                    for c in range(4):
                        nc.tensor.matmul(
                            pv,
                            xkb[:, c, mi, kc * 128 : (kc + 1) * 128],
                            wvb[:, c, dvh * 512 : (dvh + 1) * 512],
                            start=(c == 0),
                            stop=(c == 3),
                        )
                    v4 = vp2[kc].rearrange("p (h c) -> p h c", h=H)
                    ecopy(
                        eng if eng is not None else pick2(612, 658),
                        v4[:, dvh * 8 : (dvh + 1) * 8, off : off + 64],
                        pv.rearrange("p (h d) -> p h d", h=8),
                    )

                p3_units = [
                    (mi, kc, dvh)
                    for dvh in range(2)
                    for kc in range(KC)
                    for mi in range(M)
                ]

                with (
                    tc.tile_pool(name="ppj", bufs=3, space="PSUM") as ppj,
                    tc.tile_pool(name="ppv", bufs=2, space="PSUM") as ppv1,
                ):
                    for s in range(2):
                        for hp8 in range(HP):
                            proj_pair(ppj, hp8, s, wk8, xk8, kk[hp8])
                    # P1 with the 16 dvh=0 P3 units woven in
                    for hp8 in range(HP):
                        proj_pair(ppj, hp8, 0, wq8, xq8, qq[hp8])
                        p3_unit(p3_units[2 * hp8], ppv1)
                        proj_pair(ppj, hp8, 1, wq8, xq8, qq[hp8])
                        p3_unit(p3_units[2 * hp8 + 1], ppv1)

            # ============ attention (software-pipelined head loop) ============
            with (
                tc.tile_pool(name="exp", bufs=8) as expp,
                tc.tile_pool(name="sump", bufs=3) as sump,
                tc.tile_pool(name="repp", bufs=5) as repp,
                tc.tile_pool(name="opop", bufs=4) as opop,
                tc.tile_pool(name="pse", bufs=2, space="PSUM") as pse,
                tc.tile_pool(name="pso", bufs=1, space="PSUM") as pso,
                tc.tile_pool(name="ppv2", bufs=2, space="PSUM") as ppv2,
            ):

                def emit_e(h):
                    """E DR matmuls (two 64-partition s-passes) + mask-bias
                    matmul + exp -> masked em"""
                    g, h2 = h // 2, h % 2
                    sl = slice(h2 * 64, (h2 + 1) * 64)
                    em = []
                    for p in range(2):
                        pe2 = pse.tile([128, 2, N], F32, name="pe", tag="pe")
                        for half in range(2):
                            kc = 2 * p + half
                            for s in range(2):
                                nc.tensor.matmul(
                                    pe2[:, half, :],
                                    kk[g][sl, s, :, kc * 128 : (kc + 1) * 128],
                                    qq[g][sl, s, :, :],
                                    start=(s == 0),
                                    stop=False,
                                    perf_mode=DR,
                                )
                            nc.tensor.matmul(
                                pe2[:, half, :],
                                id8,
                                mb8[:, :, kc, :],
                                start=False,
                                stop=True,
                                perf_mode=DR,
                            )
                        ex2 = expp.tile([128, 2, N], BF, name="ex", tag="ex")
                        nc.scalar.activation(
                            ex2.rearrange("p t n -> p (t n)"),
                            pe2.rearrange("p t n -> p (t n)"),
                            AF.Exp,
                            scale=ESCALE,
                        )
                        em.append(ex2)
                    return em

                def emit_sums(h, em):
                    """bf16 tree-adds (DVE) + Pool all-reduce"""
                    t01 = sump.tile([128, 2, N], BF, name="t01", tag="t01")
                    nc.vector.tensor_add(t01, em[0], em[1])
                    s_all = sump.tile([128, N], BF, name="s_all", tag="s_all")
                    nc.vector.tensor_add(s_all, t01[:, 0, :], t01[:, 1, :])
                    s_red = sump.tile([128, N], BF, name="s_red", tag="s_red")
                    nc.gpsimd.partition_all_reduce(
                        s_red, s_all, channels=128, reduce_op=bass_isa.ReduceOp.add
                    )
                    return s_red

                def emit_recip(s_red):
                    rep = repp.tile([128, N], BF, name="rep", tag="rep")
                    with nc.allow_low_precision(reason="softmax 1/sum"):
                        nc.vector.reciprocal(rep, s_red)
                    return rep

                def emit_o(h, em):
                    """O matmuls into a 2-bank psum; evac alternates ACT/DVE"""
                    po2 = pso.tile([128, 2, N], F32, name="po", tag="po")
                    for mp in range(2):
                        for kc in range(KC):
                            nc.tensor.matmul(
                                po2[:, mp, :],
                                vp2[kc][:, h * 256 + mp * 128 : h * 256 + (mp + 1) * 128],
                                em[kc // 2][:, kc % 2, :],
                                start=(kc == 0),
                                stop=(kc == KC - 1),
                            )
                    opo = opop.tile([128, 2, N], BF, name="opo", tag="opo")
                    ecopy(nc.scalar, opo.rearrange("p t n -> p (t n)"),
                          po2.rearrange("p t n -> p (t n)"))
                    return opo

                def emit_norm(h, opo, rep, dve_all=False):
                    """normalize O into OT tiles: 2 muls on DVE, 2 on Pool"""
                    hp, hs = h // 2, h % 2
                    for i, (mp, s) in enumerate(
                        ((0, 0), (0, 1), (1, 0), (1, 1))
                    ):
                        eng = nc.vector if (dve_all or i % 2 == 0) else nc.gpsimd
                        eng.tensor_mul(
                            ot[hp][hs * 64 : (hs + 1) * 64, 2 * mp + s, :],
                            opo[s * 64 : (s + 1) * 64, mp, :],
                            rep[s * 64 : (s + 1) * 64, :],
                        )

                em_q, rep_q, opo_q = {}, {}, {}
                # dvh=1 P3 units woven 2/head through heads 0..7 so all of
                # them are emitted before O(8) consumes their vp2 columns.
                # O lags E by 3 heads so the last O-chains (and their evac/
                # norm engine work) ride the P7 warm stretch.
                p3i = 16
                for h in range(H):
                    if h >= 3:
                        opo_q[h - 3] = emit_o(h - 3, em_q.pop(h - 3))
                    em_q[h] = emit_e(h)
                    # all 16 dvh=1 units must be emitted before O(8) at
                    # iteration 11; spread them ~1.5/head over heads 0..10
                    while p3i < min(32, 16 + (16 * (h + 1) + 10) // 11):
                        p3_unit(p3_units[p3i], ppv2,
                                eng=nc.scalar if p3i % 2 == 0 else nc.vector)
                        p3i += 1
                    if h >= 1:
                        rep_q[h - 1] = emit_recip(emit_sums(h - 1, em_q[h - 1]))
                    if h >= 4:
                        emit_norm(h - 4, opo_q.pop(h - 4), rep_q.pop(h - 4))
                # epilogue: O(13..15) + norms 12..15 overlap the P7 warm units
                rep_q[15] = emit_recip(emit_sums(15, em_q[15]))
                opo_q[13] = emit_o(13, em_q.pop(13))
                emit_norm(12, opo_q.pop(12), rep_q.pop(12))
                opo_q[14] = emit_o(14, em_q.pop(14))
                emit_norm(13, opo_q.pop(13), rep_q.pop(13))
                opo_q[15] = emit_o(15, em_q.pop(15))
                emit_norm(14, opo_q.pop(14), rep_q.pop(14))
                emit_norm(15, opo_q.pop(15), rep_q.pop(15), dve_all=True)

            # ============ P7: output projection (bf16) ============
            # The first four psum groups run contraction chunks hp=0..5
            # before any hp>=6 step, so the PE keeps streaming while the
            # last two heads' normalized OT tiles are still being written.
            with (
                tc.tile_pool(name="outst", bufs=4) as outstp,
                tc.tile_pool(name="psf", bufs=4, space="PSUM") as psf,
            ):
                units = [(mi, ncc) for mi in range(M) for ncc in range(NC)]

                def p7_mms(pf, mi, ncc, dvh, hps, first, last):
                    for hp in hps:
                        nc.tensor.matmul(
                            pf,
                            ot[hp][:, mi, ncc * 128 : (ncc + 1) * 128],
                            wob[:, hp, dvh * 512 : (dvh + 1) * 512],
                            start=(hp == first),
                            stop=(hp == last),
                        )

                def p7_finish(u, pf0, pf1):
                    mi, ncc = u
                    ost = outstp.tile([128, 2, 512], F32, name="ost", tag="ost")
                    ecopy(pick2(612, 658), ost[:, 0, :], pf0)
                    ecopy(pick2(612, 658), ost[:, 1, :], pf1)
                    nc.sync.dma_start(
                        out_d.ap()[ncc * 128 : (ncc + 1) * 128, mi, :],
                        ost.rearrange("p a b -> p (a b)"),
                    )

                # warm stretch: 4 psum groups of hp0..5 for the first 2 units
                warm = []
                for u in units[:4]:
                    mi, ncc = u
                    pfs = []
                    for dvh in range(2):
                        pf = psf.tile([128, 512], F32, name="pf", tag="pf")
                        p7_mms(pf, mi, ncc, dvh, range(6), 0, HP - 1)
                        pfs.append(pf)
                    warm.append((u, pfs))
                for u, pfs in warm:
                    mi, ncc = u
                    for dvh in range(2):
                        p7_mms(pfs[dvh], mi, ncc, dvh, range(6, HP), 0, HP - 1)
                    p7_finish(u, *pfs)
                for u in units[4:-2]:
                    mi, ncc = u
                    pfs = []
                    for dvh in range(2):
                        pf = psf.tile([128, 512], F32, name="pf", tag="pf")
                        p7_mms(pf, mi, ncc, dvh, range(HP), 0, HP - 1)
                        pfs.append(pf)
                    p7_finish(u, *pfs)
                # last two units: per-half evac/DMA pipeline so the final
                # evacuation and store overlap the preceding unit's matmuls;
                # the very last half is split in two quarters so the final
                # serialized evac+DMA chain is as short as possible
                for u in units[-2:]:
                    last = u == units[-1]
                    mi, ncc = u
                    for dvh in range(2):
                        pf = psf.tile([128, 512], F32, name="pf", tag="pf")
                        p7_mms(pf, mi, ncc, dvh, range(HP), 0, HP - 1)
                        pieces = (
                            ((0, 384), (384, 512))
                            if (last and dvh == 1)
                            else ((0, 512),)
                        )
                        for lo, hi in pieces:
                            osh = outstp.tile([128, 512], F32, name="osh", tag="osh")
                            eng = nc.scalar if lo == 0 else nc.vector
                            ecopy(eng, osh[:, 0 : hi - lo], pf[:, lo:hi])
                            nc.sync.dma_start(
                                out_d.ap()[
                                    ncc * 128 : (ncc + 1) * 128,
                                    mi,
                                    dvh * 512 + lo : dvh * 512 + hi,
                                ],
                                osh[:, 0 : hi - lo],
                            )

    nc.compile()
    return nc


_NC_CACHE = None


def _get_nc():
    global _NC_CACHE
    if _NC_CACHE is None:
        _NC_CACHE = build_nc()
    return _NC_CACHE


def _dr_pack(a):
    """[512, ...] rows dq = c*256 + i*128 + p -> [128 = p, 2 = c, 2 = i, ...]"""
    s = a.shape[1:]
    return a.reshape(2, 2, 128, *s).transpose(2, 0, 1, *range(3, 3 + len(s)))


def kernel(Q, K, mask, w_q, w_k, w_v, w_o):
    from concourse.bass_utils import run_bass_kernel_spmd

    Q = np.asarray(Q, dtype=np.float32)
    K = np.asarray(K, dtype=np.float32)
    mask = np.asarray(mask)
    w_q = np.asarray(w_q, dtype=np.float32)
    w_k = np.asarray(w_k, dtype=np.float32)
    w_v = np.asarray(w_v, dtype=np.float32)
    w_o = np.asarray(w_o, dtype=np.float32)

    wq8 = np.ascontiguousarray(_dr_pack(w_q * WS)).astype(NPF8)
    wk8 = np.ascontiguousarray(_dr_pack(w_k * WS)).astype(NPF8)
    wvb = np.ascontiguousarray(
        w_v.reshape(4, 128, DV).transpose(1, 0, 2)
    ).astype(NPBF)
    wob = np.ascontiguousarray(
        w_o.reshape(HP, 128, DV).transpose(1, 0, 2)
    ).astype(NPBF)
    id8 = np.zeros((128, 2, 128), NPF8E5)
    id8[:, 0, :] = (np.eye(128) * 1024.0).astype(NPF8E5)

    in_maps = []
    for b in range(B):
        qt = np.ascontiguousarray(Q[b].transpose(2, 1, 0))   # [DQ, M, N]
        kt = np.ascontiguousarray(K[b].transpose(2, 1, 0))   # [DK, M, KN]
        mb = np.zeros((128, 2, KC, N), NPF8E5)
        # mask[b] is [N, KN]; mb[p, 0, kc, n] = MBIAS where mask[n, kc*128+p]==0
        mt = mask[b].T.reshape(KC, 128, N).transpose(1, 0, 2)  # [p, kc, n]
        mb[:, 0, :, :] = (MBIAS * (1 - mt)).astype(NPF8E5)
        in_maps.append(
            {
                "QT8": np.ascontiguousarray(_dr_pack(qt)).astype(NPF8),
                "KT8": np.ascontiguousarray(_dr_pack(kt)).astype(NPF8),
                "KTB": np.ascontiguousarray(
                    kt.reshape(4, 128, M, KN).transpose(1, 0, 2, 3)
                ).astype(NPBF),
                "MB8": mb,
                "ID8": id8,
                "WQ8": wq8,
                "WK8": wk8,
                "WVB": wvb,
                "WOB": wob,
            }
        )

    nc = _get_nc()
    r = run_bass_kernel_spmd(nc, in_maps, core_ids=list(range(B)), trace=False)
    return np.stack([r.results[b]["out"] for b in range(B)], axis=0)


if __name__ == "__main__":
    rng = np.random.default_rng(0)
    inputs = {
        "Q": rng.standard_normal((B, N, M, DQ), dtype=np.float32),
        "K": rng.standard_normal((B, KN, M, DK), dtype=np.float32),
        "mask": rng.integers(0, 2, (B, N, KN)).astype(np.int32),
        "w_q": (rng.standard_normal((DQ, DV), dtype=np.float32) * 0.02),
        "w_k": (rng.standard_normal((DK, DV), dtype=np.float32) * 0.02),
        "w_v": (rng.standard_normal((DK, DV), dtype=np.float32) * 0.02),
        "w_o": (rng.standard_normal((DV, DV), dtype=np.float32) * 0.02),
    }
    out = kernel(**inputs)
    print("out", out.shape, out.dtype, float(np.abs(out).max()))


# revision 53
# speedup vs baseline: 1.1406x; 1.0073x over previous
"""EquiMHA Trainium2 kernel.

Data-parallel over batch B=8 across the 8 NeuronCores (one batch element per
core, weights replicated, no collectives).

Per-core computation for batch b (N=512, M=4, KN=512, DQ=DK=512, DV=1024,
H=16, D=64):
  Qp = Q[b] @ w_q, Kp = K[b] @ w_k, Vp = K[b] @ w_v
  E[h,n,k] = sum_{m,d} Qp[n,m,h*64+d] Kp[k,m,h*64+d] / 32
  A = masked_softmax(E)        (max-subtraction skipped: |E|/32 <= ~2, and the
                                max cancels exactly up to the +eps term)
  O[n,m,h*64+d] = sum_k A[h,n,k] Vp[k,m,h*64+d]
  out = O @ w_o

Precision strategy (tolerance is 2e-2; measured pipeline error ~9e-3):
  - Q/K-side projections run in fp8e4m3 with DoubleRow perf mode (2x PE
    rate, 256-deep contraction per pass); stationary tiles are 128 columns
    wide (a head PAIR), which halves the projection matmul count vs 64-col.
  - The projected qpp/kpp head tiles are themselves stored in fp8e4m3, so
    the E (scores) matmuls also run DoubleRow at 2x rate with the full
    256-deep (m,d) contraction in a single pass: 1 matmul per 128-k chunk.
  - The mask is folded into the E PSUM on the PE: one extra DoubleRow
    matmul per k-chunk with an fp8e5m2 identity stationary (value 1024) and
    an fp8e5m2 mask-bias moving operand (-4096 where masked) adds -2^22 to
    masked logits; after the exp scale that's -32, so exp()==~2e-14==0.
    This removes all 64 DVE mask-multiplies from the softmax chain.
  - V path (P3), A@V (O), and the output projection (P7) stay bf16: fp8
    anywhere on the V/O path adds ~3-4e-2 relative error (V-side errors are
    not softmax-damped), which blows the 2e-2 budget.

Schedule strategy (the cost model charges matmuls by output free size and
models the PE p-state ramp, so the kernel keeps the PE busy end to end):
  - ~44 zero warm-up matmuls fill the initial input-DMA window so the PE
    p-state is fully ramped when the first projection lands.
  - Input DMAs are sliced (per m chunk) and ordered to match first use.
  - P3 (Vp, bf16) is split: 16 units woven through the P1 era, 16 through
    the attention head loop, so the PE never idles while the ACT/DVE
    engines drain the PSUM-evacuation backlog.
  - Attention head loop is software-pipelined (O lags E by 3 heads so the
    tail rides the P7 warm stretch) with a static engine assignment (Pool
    cannot touch PSUM): exp and the O-psum evacuation on ACT (ACT's queue
    is short, so the evac lands in time for the 1-buf O-psum ring);
    partial-sum adds, reciprocal and 2 of 4 norm muls on DVE;
    partition_all_reduce plus the other 2 norm muls on Pool.
"""

import numpy as np
import ml_dtypes

import concourse.bacc as bacc
import concourse.mybir as mybir
import concourse.tile as tile

F32 = mybir.dt.float32
F8 = mybir.dt.float8e4
F8E5 = mybir.dt.float8e5
BF = mybir.dt.bfloat16
AF = mybir.ActivationFunctionType
DR = mybir.MatmulPerfMode.DoubleRow

NPF8 = ml_dtypes.float8_e4m3
NPF8E5 = ml_dtypes.float8_e5m2
NPBF = ml_dtypes.bfloat16

B, N, M, KN = 8, 512, 4, 512
DQ, DK, DV, H = 512, 512, 1024, 16
D = DV // H
HP = H // 2          # head pairs (P7 contraction chunks)
KC = KN // 128       # k chunks
NC = N // 128        # n chunks
WS = 64.0            # host pre-scale for fp8 weights
SCALE = 1.0 / 32.0   # 1/sqrt(DV)
ESCALE = SCALE / (WS * WS)  # fused into exp
MBIAS = -4096.0      # fp8e5 mask bias; with ID 1024 adds -32 to the logits
NWARM = 44           # p-state warm-up matmuls


def build_nc():
    nc = bacc.Bacc("TRN2", target_bir_lowering=False, debug=False, num_devices=8)

    qt8_d = nc.dram_tensor("QT8", [128, 2, 2, M, N], F8, kind="ExternalInput")
    kt8_d = nc.dram_tensor("KT8", [128, 2, 2, M, KN], F8, kind="ExternalInput")
    ktb_d = nc.dram_tensor("KTB", [128, 4, M, KN], BF, kind="ExternalInput")
    mb8_d = nc.dram_tensor("MB8", [128, 2, KC, N], F8E5, kind="ExternalInput")
    id8_d = nc.dram_tensor("ID8", [128, 2, 128], F8E5, kind="ExternalInput")
    wq8_d = nc.dram_tensor("WQ8", [128, 2, 2, DV], F8, kind="ExternalInput")
    wk8_d = nc.dram_tensor("WK8", [128, 2, 2, DV], F8, kind="ExternalInput")
    wvb_d = nc.dram_tensor("WVB", [128, 4, DV], BF, kind="ExternalInput")
    wob_d = nc.dram_tensor("WOB", [128, 8, DV], BF, kind="ExternalInput")
    out_d = nc.dram_tensor("out", [N, M, DV], F32, kind="ExternalOutput")

    import concourse.bass_isa as bass_isa

    # --- greedy projected-load balancer for evac copies (ns estimates) ---
    eng_ns = {"act": 0.0, "dve": 0.0}

    def pick2(act_cost, dve_cost):
        if eng_ns["act"] + act_cost <= eng_ns["dve"] + dve_cost:
            eng_ns["act"] += act_cost
            return nc.scalar
        eng_ns["dve"] += dve_cost
        return nc.vector

    def ecopy(eng, dst, src):
        if eng is nc.scalar:
            nc.scalar.copy(dst, src)
        else:
            eng.tensor_copy(dst, src)

    with tile.TileContext(nc) as tc:
        with tc.tile_pool(name="persist", bufs=1) as persist:
            vp2 = [persist.tile([128, M * DV], BF, name=f"vp2_{kc}") for kc in range(KC)]
            # qq/kk [(h2,d) 128, s 2, mp 2, n] fp8: PSUM-native layout so each
            # projection evacuation is one full-128-partition copy; E slices
            # the h2 half and accumulates over s in two 64-partition DR passes
            qq = [persist.tile([128, 2, 2, N], F8, name=f"qq{g}") for g in range(HP)]
            kk = [persist.tile([128, 2, 2, KN], F8, name=f"kk{g}") for g in range(HP)]
            ot = [persist.tile([128, M, N], BF, name=f"ot{hp}") for hp in range(HP)]
            wob = persist.tile([128, HP, DV], BF, name="wob")
            mb8 = persist.tile([128, 2, KC, N], F8E5, name="mb8")
            id8 = persist.tile([128, 2, 128], F8E5, name="id8")
            xkb = persist.tile([128, 4, M, KN], BF, name="xkb")
            wvb = persist.tile([128, 4, DV], BF, name="wvb")
            # wz lives in the persistent pool so the input DMAs' destination
            # tiles never overlap it (a scoped pool here made the first xk8
            # DMA wait on all warm-up matmuls through a WAR dependency)
            wz = persist.tile([128, 128], BF, name="wz")

            # ---- p-state warm-up: keep the PE busy during the input DMAs ----
            with tc.tile_pool(name="warmps", bufs=1, space="PSUM") as warmpsp:
                nc.vector.memzero(wz)
                wps = warmpsp.tile([128, 128], F32, name="wps")
                for _ in range(NWARM):
                    nc.tensor.matmul(wps, wz, wz, start=True, stop=True)

            # ============ P1/P2 projections (fp8 DR, head-pair stationary) ====
            with (
                tc.tile_pool(name="xk8", bufs=1) as xk8p,
                tc.tile_pool(name="w8", bufs=1) as w8p,
            ):
                xk8 = xk8p.tile([128, 2, 2, M, KN], F8, name="xk8")
                wk8 = w8p.tile([128, 2, 2, DV], F8, name="wk8")
                xq8 = xk8p.tile([128, 2, 2, M, N], F8, name="xq8")
                wq8 = w8p.tile([128, 2, 2, DV], F8, name="wq8")
                # DMA order tracks PE consumption order (the model's DMA
                # engines are a single serial resource)
                nc.sync.dma_start(wk8[:, :, :, 0:512], wk8_d.ap()[:, :, :, 0:512])
                for m in (0, 2):
                    nc.sync.dma_start(xk8[:, :, :, m, :], kt8_d.ap()[:, :, :, m, :])
                nc.sync.dma_start(wk8[:, :, :, 512:1024], wk8_d.ap()[:, :, :, 512:1024])
                for m in (1, 3):
                    nc.sync.dma_start(xk8[:, :, :, m, :], kt8_d.ap()[:, :, :, m, :])
                for c in range(4):
                    nc.sync.dma_start(wvb[:, c, :], wvb_d.ap()[:, c, :])
                    nc.sync.dma_start(xkb[:, c, :, :], ktb_d.ap()[:, c, :, :])
                nc.sync.dma_start(wq8, wq8_d.ap())
                for m in (0, 2, 1, 3):
                    nc.sync.dma_start(xq8[:, :, :, m, :], qt8_d.ap()[:, :, :, m, :])
                nc.sync.dma_start(id8, id8_d.ap())
                nc.sync.dma_start(mb8, mb8_d.ap())
                nc.sync.dma_start(wob, wob_d.ap())

                def proj_pair(pool, hp8, s, w_sb, x_sb, dst):
                    """One head-pair, one m-parity: psum [128=(h2,d), mp, n];
                    single full-partition evacuation into dst[:, s, :, :]"""
                    pq = pool.tile([128, 2, N], F32, name="pq", tag="pq")
                    for mp in range(2):
                        m = 2 * mp + s
                        for c in range(2):
                            nc.tensor.matmul(
                                pq[:, mp, :],
                                w_sb[:, c, :, hp8 * 128 : (hp8 + 1) * 128],
                                x_sb[:, c, :, m, :],
                                start=(c == 0),
                                stop=(c == 1),
                                perf_mode=DR,
                            )
                    ecopy(pick2(1038, 1192), dst[:, s, :, :], pq)

                def p3_unit(u, pool, eng=None):
                    mi, kc, dvh = u
                    off = mi * 64
                    pv = pool.tile([128, 512], F32, name="pv", tag="pv")
                    for c in range(4):
                        nc.tensor.matmul(
                            pv,
                            xkb[:, c, mi, kc * 128 : (kc + 1) * 128],
                            wvb[:, c, dvh * 512 : (dvh + 1) * 512],
                            start=(c == 0),
                            stop=(c == 3),
                        )
                    v4 = vp2[kc].rearrange("p (h c) -> p h c", h=H)
                    ecopy(
                        eng if eng is not None else pick2(612, 658),
                        v4[:, dvh * 8 : (dvh + 1) * 8, off : off + 64],
                        pv.rearrange("p (h d) -> p h d", h=8),
                    )

                p3_units = [
                    (mi, kc, dvh)
                    for dvh in range(2)
                    for kc in range(KC)
                    for mi in range(M)
                ]

                with (
                    tc.tile_pool(name="ppj", bufs=3, space="PSUM") as ppj,
                    tc.tile_pool(name="ppv", bufs=2, space="PSUM") as ppv1,
                ):
                    for s in range(2):
                        for hp8 in range(HP):
                            proj_pair(ppj, hp8, s, wk8, xk8, kk[hp8])
                    # P1 with the 16 dvh=0 P3 units woven in
                    for hp8 in range(HP):
                        proj_pair(ppj, hp8, 0, wq8, xq8, qq[hp8])
                        p3_unit(p3_units[2 * hp8], ppv1)
                        proj_pair(ppj, hp8, 1, wq8, xq8, qq[hp8])
                        p3_unit(p3_units[2 * hp8 + 1], ppv1)

            # ============ attention (software-pipelined head loop) ============
            with (
                tc.tile_pool(name="exp", bufs=8) as expp,
                tc.tile_pool(name="sump", bufs=3) as sump,
                tc.tile_pool(name="repp", bufs=5) as repp,
                tc.tile_pool(name="opop", bufs=4) as opop,
                tc.tile_pool(name="pse", bufs=2, space="PSUM") as pse,
                tc.tile_pool(name="pso", bufs=1, space="PSUM") as pso,
                tc.tile_pool(name="ppv2", bufs=2, space="PSUM") as ppv2,
            ):

                def emit_e(h):
                    """E DR matmuls (two 64-partition s-passes) + mask-bias
                    matmul + exp -> masked em"""
                    g, h2 = h // 2, h % 2
                    sl = slice(h2 * 64, (h2 + 1) * 64)
                    em = []
                    for p in range(2):
                        pe2 = pse.tile([128, 2, N], F32, name="pe", tag="pe")
                        for half in range(2):
                            kc = 2 * p + half
                            for s in range(2):
                                nc.tensor.matmul(
                                    pe2[:, half, :],
                                    kk[g][sl, s, :, kc * 128 : (kc + 1) * 128],
                                    qq[g][sl, s, :, :],
                                    start=(s == 0),
                                    stop=False,
                                    perf_mode=DR,
                                )
                            nc.tensor.matmul(
                                pe2[:, half, :],
                                id8,
                                mb8[:, :, kc, :],
                                start=False,
                                stop=True,
                                perf_mode=DR,
                            )
                        ex2 = expp.tile([128, 2, N], BF, name="ex", tag="ex")
                        nc.scalar.activation(
                            ex2.rearrange("p t n -> p (t n)"),
                            pe2.rearrange("p t n -> p (t n)"),
                            AF.Exp,
                            scale=ESCALE,
                        )
                        em.append(ex2)
                    return em

                def emit_sums(h, em):
                    """bf16 tree-adds (DVE) + Pool all-reduce"""
                    t01 = sump.tile([128, 2, N], BF, name="t01", tag="t01")
                    nc.vector.tensor_add(t01, em[0], em[1])
                    s_all = sump.tile([128, N], BF, name="s_all", tag="s_all")
                    nc.vector.tensor_add(s_all, t01[:, 0, :], t01[:, 1, :])
                    s_red = sump.tile([128, N], BF, name="s_red", tag="s_red")
                    nc.gpsimd.partition_all_reduce(
                        s_red, s_all, channels=128, reduce_op=bass_isa.ReduceOp.add
                    )
                    return s_red

                def emit_recip(s_red):
                    rep = repp.tile([128, N], BF, name="rep", tag="rep")
                    with nc.allow_low_precision(reason="softmax 1/sum"):
                        nc.vector.reciprocal(rep, s_red)
                    return rep

                def emit_o(h, em):
                    """O matmuls into a 2-bank psum; evac alternates ACT/DVE"""
                    po2 = pso.tile([128, 2, N], F32, name="po", tag="po")
                    for mp in range(2):
                        for kc in range(KC):
                            nc.tensor.matmul(
                                po2[:, mp, :],
                                vp2[kc][:, h * 256 + mp * 128 : h * 256 + (mp + 1) * 128],
                                em[kc // 2][:, kc % 2, :],
                                start=(kc == 0),
                                stop=(kc == KC - 1),
                            )
                    opo = opop.tile([128, 2, N], BF, name="opo", tag="opo")
                    ecopy(nc.scalar, opo.rearrange("p t n -> p (t n)"),
                          po2.rearrange("p t n -> p (t n)"))
                    return opo

                def emit_norm(h, opo, rep, dve_all=False):
                    """normalize O into OT tiles: 2 muls on DVE, 2 on Pool"""
                    hp, hs = h // 2, h % 2
                    for i, (mp, s) in enumerate(
                        ((0, 0), (0, 1), (1, 0), (1, 1))
                    ):
                        eng = nc.vector if (dve_all or i % 2 == 0) else nc.gpsimd
                        eng.tensor_mul(
                            ot[hp][hs * 64 : (hs + 1) * 64, 2 * mp + s, :],
                            opo[s * 64 : (s + 1) * 64, mp, :],
                            rep[s * 64 : (s + 1) * 64, :],
                        )

                em_q, rep_q, opo_q = {}, {}, {}
                # dvh=1 P3 units woven ~1.5/head through heads 0..10 so all
                # are emitted before O(8) consumes their vp2 columns at
                # iteration 11. O lags E by 3 heads so the last O-chains
                # (and their evac/norm engine work) ride the P7 warm stretch.
                p3i = 16
                for h in range(H):
                    if h >= 3:
                        opo_q[h - 3] = emit_o(h - 3, em_q.pop(h - 3))
                    em_q[h] = emit_e(h)
                    while p3i < min(32, 16 + (16 * (h + 1) + 10) // 11):
                        p3_unit(p3_units[p3i], ppv2,
                                eng=nc.scalar if p3i % 2 == 0 else nc.vector)
                        p3i += 1
                    if h >= 1:
                        rep_q[h - 1] = emit_recip(emit_sums(h - 1, em_q[h - 1]))
                    if h >= 4:
                        emit_norm(h - 4, opo_q.pop(h - 4), rep_q.pop(h - 4))
                # epilogue: O(13..15) + norms 12..15 overlap the P7 warm units
                rep_q[15] = emit_recip(emit_sums(15, em_q[15]))
                opo_q[13] = emit_o(13, em_q.pop(13))
                emit_norm(12, opo_q.pop(12), rep_q.pop(12))
                opo_q[14] = emit_o(14, em_q.pop(14))
                emit_norm(13, opo_q.pop(13), rep_q.pop(13))
                opo_q[15] = emit_o(15, em_q.pop(15))
                emit_norm(14, opo_q.pop(14), rep_q.pop(14))
                emit_norm(15, opo_q.pop(15), rep_q.pop(15), dve_all=True)

            # ============ P7: output projection (bf16) ============
            # The first four psum groups run contraction chunks hp=0..5
            # before any hp>=6 step, so the PE keeps streaming while the
            # last two heads' normalized OT tiles are still being written.
            with (
                tc.tile_pool(name="outst", bufs=4) as outstp,
                tc.tile_pool(name="psf", bufs=4, space="PSUM") as psf,
            ):
                units = [(mi, ncc) for mi in range(M) for ncc in range(NC)]

                def p7_mms(pf, mi, ncc, dvh, hps, first, last):
                    for hp in hps:
                        nc.tensor.matmul(
                            pf,
                            ot[hp][:, mi, ncc * 128 : (ncc + 1) * 128],
                            wob[:, hp, dvh * 512 : (dvh + 1) * 512],
                            start=(hp == first),
                            stop=(hp == last),
                        )

                def p7_finish(u, pf0, pf1):
                    mi, ncc = u
                    ost = outstp.tile([128, 2, 512], F32, name="ost", tag="ost")
                    ecopy(pick2(612, 658), ost[:, 0, :], pf0)
                    ecopy(pick2(612, 658), ost[:, 1, :], pf1)
                    nc.sync.dma_start(
                        out_d.ap()[ncc * 128 : (ncc + 1) * 128, mi, :],
                        ost.rearrange("p a b -> p (a b)"),
                    )

                # warm stretch: 4 psum groups of hp0..5 for the first 2 units
                warm = []
                for u in units[:4]:
                    mi, ncc = u
                    pfs = []
                    for dvh in range(2):
                        pf = psf.tile([128, 512], F32, name="pf", tag="pf")
                        p7_mms(pf, mi, ncc, dvh, range(6), 0, HP - 1)
                        pfs.append(pf)
                    warm.append((u, pfs))
                for u, pfs in warm:
                    mi, ncc = u
                    for dvh in range(2):
                        p7_mms(pfs[dvh], mi, ncc, dvh, range(6, HP), 0, HP - 1)
                    p7_finish(u, *pfs)
                for u in units[4:-2]:
                    mi, ncc = u
                    pfs = []
                    for dvh in range(2):
                        pf = psf.tile([128, 512], F32, name="pf", tag="pf")
                        p7_mms(pf, mi, ncc, dvh, range(HP), 0, HP - 1)
                        pfs.append(pf)
                    p7_finish(u, *pfs)
                # last two units: per-half evac/DMA pipeline so the final
                # evacuation and store overlap the preceding unit's matmuls;
                # the very last half is split in two quarters so the final
                # serialized evac+DMA chain is as short as possible
                for u in units[-2:]:
                    last = u == units[-1]
                    mi, ncc = u
                    for dvh in range(2):
                        pf = psf.tile([128, 512], F32, name="pf", tag="pf")
                        p7_mms(pf, mi, ncc, dvh, range(HP), 0, HP - 1)
                        pieces = (
                            ((0, 384), (384, 512))
                            if (last and dvh == 1)
                            else ((0, 512),)
                        )
                        for lo, hi in pieces:
                            osh = outstp.tile([128, 512], F32, name="osh", tag="osh")
                            eng = nc.scalar if lo == 0 else nc.vector
                            ecopy(eng, osh[:, 0 : hi - lo], pf[:, lo:hi])
                            nc.sync.dma_start(
                                out_d.ap()[
                                    ncc * 128 : (ncc + 1) * 128,
                                    mi,
                                    dvh * 512 + lo : dvh * 512 + hi,
                                ],
                                osh[:, 0 : hi - lo],
                            )

    nc.compile()
    return nc


_NC_CACHE = None


def _get_nc():
    global _NC_CACHE
    if _NC_CACHE is None:
        _NC_CACHE = build_nc()
    return _NC_CACHE


def _dr_pack(a):
    """[512, ...] rows dq = c*256 + i*128 + p -> [128 = p, 2 = c, 2 = i, ...]"""
    s = a.shape[1:]
    return a.reshape(2, 2, 128, *s).transpose(2, 0, 1, *range(3, 3 + len(s)))


def kernel(Q, K, mask, w_q, w_k, w_v, w_o):
    from concourse.bass_utils import run_bass_kernel_spmd

    Q = np.asarray(Q, dtype=np.float32)
    K = np.asarray(K, dtype=np.float32)
    mask = np.asarray(mask)
    w_q = np.asarray(w_q, dtype=np.float32)
    w_k = np.asarray(w_k, dtype=np.float32)
    w_v = np.asarray(w_v, dtype=np.float32)
    w_o = np.asarray(w_o, dtype=np.float32)

    wq8 = np.ascontiguousarray(_dr_pack(w_q * WS)).astype(NPF8)
    wk8 = np.ascontiguousarray(_dr_pack(w_k * WS)).astype(NPF8)
    wvb = np.ascontiguousarray(
        w_v.reshape(4, 128, DV).transpose(1, 0, 2)
    ).astype(NPBF)
    wob = np.ascontiguousarray(
        w_o.reshape(HP, 128, DV).transpose(1, 0, 2)
    ).astype(NPBF)
    id8 = np.zeros((128, 2, 128), NPF8E5)
    id8[:, 0, :] = (np.eye(128) * 1024.0).astype(NPF8E5)

    in_maps = []
    for b in range(B):
        qt = np.ascontiguousarray(Q[b].transpose(2, 1, 0))   # [DQ, M, N]
        kt = np.ascontiguousarray(K[b].transpose(2, 1, 0))   # [DK, M, KN]
        mb = np.zeros((128, 2, KC, N), NPF8E5)
        # mask[b] is [N, KN]; mb[p, 0, kc, n] = MBIAS where mask[n, kc*128+p]==0
        mt = mask[b].T.reshape(KC, 128, N).transpose(1, 0, 2)  # [p, kc, n]
        mb[:, 0, :, :] = (MBIAS * (1 - mt)).astype(NPF8E5)
        in_maps.append(
            {
                "QT8": np.ascontiguousarray(_dr_pack(qt)).astype(NPF8),
                "KT8": np.ascontiguousarray(_dr_pack(kt)).astype(NPF8),
                "KTB": np.ascontiguousarray(
                    kt.reshape(4, 128, M, KN).transpose(1, 0, 2, 3)
                ).astype(NPBF),
                "MB8": mb,
                "ID8": id8,
                "WQ8": wq8,
                "WK8": wk8,
                "WVB": wvb,
                "WOB": wob,
            }
        )

    nc = _get_nc()
    r = run_bass_kernel_spmd(nc, in_maps, core_ids=list(range(B)), trace=False)
    return np.stack([r.results[b]["out"] for b in range(B)], axis=0)


if __name__ == "__main__":
    rng = np.random.default_rng(0)
    inputs = {
        "Q": rng.standard_normal((B, N, M, DQ), dtype=np.float32),
        "K": rng.standard_normal((B, KN, M, DK), dtype=np.float32),
        "mask": rng.integers(0, 2, (B, N, KN)).astype(np.int32),
        "w_q": (rng.standard_normal((DQ, DV), dtype=np.float32) * 0.02),
        "w_k": (rng.standard_normal((DK, DV), dtype=np.float32) * 0.02),
        "w_v": (rng.standard_normal((DK, DV), dtype=np.float32) * 0.02),
        "w_o": (rng.standard_normal((DV, DV), dtype=np.float32) * 0.02),
    }
    out = kernel(**inputs)
    print("out", out.shape, out.dtype, float(np.abs(out).max()))


# revision 64
# speedup vs baseline: 1.1468x; 1.0055x over previous
"""EquiMHA Trainium2 kernel.

Data-parallel over batch B=8 across the 8 NeuronCores (one batch element per
core, weights replicated, no collectives).

Per-core computation for batch b (N=512, M=4, KN=512, DQ=DK=512, DV=1024,
H=16, D=64):
  Qp = Q[b] @ w_q, Kp = K[b] @ w_k, Vp = K[b] @ w_v
  E[h,n,k] = sum_{m,d} Qp[n,m,h*64+d] Kp[k,m,h*64+d] / 32
  A = masked_softmax(E)        (max-subtraction skipped: |E|/32 <= ~2, and the
                                max cancels exactly up to the +eps term)
  O[n,m,h*64+d] = sum_k A[h,n,k] Vp[k,m,h*64+d]
  out = O @ w_o

Precision strategy (tolerance is 2e-2; measured pipeline error ~9e-3):
  - Q/K-side projections run in fp8e4m3 with DoubleRow perf mode (2x PE
    rate, 256-deep contraction per pass); stationary tiles are 128 columns
    wide (a head PAIR), which halves the projection matmul count vs 64-col.
  - The projected qpp/kpp head tiles are themselves stored in fp8e4m3, so
    the E (scores) matmuls also run DoubleRow at 2x rate with the full
    256-deep (m,d) contraction in a single pass: 1 matmul per 128-k chunk.
  - The mask is folded into the E PSUM on the PE: one extra DoubleRow
    matmul per k-chunk with an fp8e5m2 identity stationary (value 1024) and
    an fp8e5m2 mask-bias moving operand (-4096 where masked) adds -2^22 to
    masked logits; after the exp scale that's -32, so exp()==~2e-14==0.
    This removes all 64 DVE mask-multiplies from the softmax chain.
  - V path (P3), A@V (O), and the output projection (P7) stay bf16: fp8
    anywhere on the V/O path adds ~3-4e-2 relative error (V-side errors are
    not softmax-damped), which blows the 2e-2 budget.

Schedule strategy (the cost model charges matmuls by output free size and
models the PE p-state ramp, so the kernel keeps the PE busy end to end):
  - ~44 zero warm-up matmuls fill the initial input-DMA window so the PE
    p-state is fully ramped when the first projection lands.
  - Input DMAs are sliced (per m chunk) and ordered to match first use.
  - P3 (Vp, bf16) is split: 16 units woven through the P1 era, 16 through
    the attention head loop, so the PE never idles while the ACT/DVE
    engines drain the PSUM-evacuation backlog.
  - Attention head loop is software-pipelined (O lags E by 3 heads so the
    tail rides the P7 warm stretch) with a static engine assignment (Pool
    cannot touch PSUM): exp and the O-psum evacuation on ACT (ACT's queue
    is short, so the evac lands in time for the 1-buf O-psum ring);
    partial-sum adds, reciprocal and 2 of 4 norm muls on DVE;
    partition_all_reduce plus the other 2 norm muls on Pool.
"""

import numpy as np
import ml_dtypes

import concourse.bacc as bacc
import concourse.mybir as mybir
import concourse.tile as tile

F32 = mybir.dt.float32
F8 = mybir.dt.float8e4
F8E5 = mybir.dt.float8e5
BF = mybir.dt.bfloat16
AF = mybir.ActivationFunctionType
DR = mybir.MatmulPerfMode.DoubleRow

NPF8 = ml_dtypes.float8_e4m3
NPF8E5 = ml_dtypes.float8_e5m2
NPBF = ml_dtypes.bfloat16

B, N, M, KN = 8, 512, 4, 512
DQ, DK, DV, H = 512, 512, 1024, 16
D = DV // H
HP = H // 2          # head pairs (P7 contraction chunks)
KC = KN // 128       # k chunks
NC = N // 128        # n chunks
WS = 64.0            # host pre-scale for fp8 weights
SCALE = 1.0 / 32.0   # 1/sqrt(DV)
ESCALE = SCALE / (WS * WS)  # fused into exp
MBIAS = -4096.0      # fp8e5 mask bias; with ID 1024 adds -32 to the logits
NWARM = 44           # p-state warm-up matmuls


def build_nc():
    nc = bacc.Bacc("TRN2", target_bir_lowering=False, debug=False, num_devices=8)

    qt8_d = nc.dram_tensor("QT8", [128, 2, 2, M, N], F8, kind="ExternalInput")
    kt8_d = nc.dram_tensor("KT8", [128, 2, 2, M, KN], F8, kind="ExternalInput")
    ktb_d = nc.dram_tensor("KTB", [128, 4, M, KN], BF, kind="ExternalInput")
    mb8_d = nc.dram_tensor("MB8", [128, 2, KC, N], F8E5, kind="ExternalInput")
    id8_d = nc.dram_tensor("ID8", [128, 2, 128], F8E5, kind="ExternalInput")
    wq8_d = nc.dram_tensor("WQ8", [128, 2, 2, DV], F8, kind="ExternalInput")
    wk8_d = nc.dram_tensor("WK8", [128, 2, 2, DV], F8, kind="ExternalInput")
    wvb_d = nc.dram_tensor("WVB", [128, 4, DV], BF, kind="ExternalInput")
    wob_d = nc.dram_tensor("WOB", [128, 8, DV], BF, kind="ExternalInput")
    out_d = nc.dram_tensor("out", [N, M, DV], F32, kind="ExternalOutput")

    import concourse.bass_isa as bass_isa

    # --- greedy projected-load balancer for evac copies (ns estimates) ---
    eng_ns = {"act": 0.0, "dve": 0.0}

    def pick2(act_cost, dve_cost):
        if eng_ns["act"] + act_cost <= eng_ns["dve"] + dve_cost:
            eng_ns["act"] += act_cost
            return nc.scalar
        eng_ns["dve"] += dve_cost
        return nc.vector

    def ecopy(eng, dst, src):
        if eng is nc.scalar:
            nc.scalar.copy(dst, src)
        else:
            eng.tensor_copy(dst, src)

    with tile.TileContext(nc) as tc:
        with tc.tile_pool(name="persist", bufs=1) as persist:
            vp2 = [persist.tile([128, M * DV], BF, name=f"vp2_{kc}") for kc in range(KC)]
            # qq/kk [(h2,d) 128, s 2, mp 2, n] fp8: PSUM-native layout so each
            # projection evacuation is one full-128-partition copy; E slices
            # the h2 half and accumulates over s in two 64-partition DR passes
            qq = [persist.tile([128, 2, 2, N], F8, name=f"qq{g}") for g in range(HP)]
            kk = [persist.tile([128, 2, 2, KN], F8, name=f"kk{g}") for g in range(HP)]
            ot = [persist.tile([128, M, N], BF, name=f"ot{hp}") for hp in range(HP)]
            wob = persist.tile([128, HP, DV], BF, name="wob")
            mb8 = persist.tile([128, 2, KC, N], F8E5, name="mb8")
            id8 = persist.tile([128, 2, 128], F8E5, name="id8")
            xkb = persist.tile([128, 4, M, KN], BF, name="xkb")
            wvb = persist.tile([128, 4, DV], BF, name="wvb")
            # wz lives in the persistent pool so the input DMAs' destination
            # tiles never overlap it (a scoped pool here made the first xk8
            # DMA wait on all warm-up matmuls through a WAR dependency)
            wz = persist.tile([128, 128], BF, name="wz")

            # ---- p-state warm-up: keep the PE busy during the input DMAs ----
            with tc.tile_pool(name="warmps", bufs=1, space="PSUM") as warmpsp:
                nc.vector.memzero(wz)
                wps = warmpsp.tile([128, 128], F32, name="wps")
                for _ in range(NWARM):
                    nc.tensor.matmul(wps, wz, wz, start=True, stop=True)

            # ============ P1/P2 projections (fp8 DR, head-pair stationary) ====
            with (
                tc.tile_pool(name="xk8", bufs=1) as xk8p,
                tc.tile_pool(name="w8", bufs=1) as w8p,
            ):
                xk8 = xk8p.tile([128, 2, 2, M, KN], F8, name="xk8")
                wk8 = w8p.tile([128, 2, 2, DV], F8, name="wk8")
                xq8 = xk8p.tile([128, 2, 2, M, N], F8, name="xq8")
                wq8 = w8p.tile([128, 2, 2, DV], F8, name="wq8")
                # DMA order tracks PE consumption order (the model's DMA
                # engines are a single serial resource)
                nc.sync.dma_start(wk8[:, :, :, 0:512], wk8_d.ap()[:, :, :, 0:512])
                for m in (0, 2):
                    nc.sync.dma_start(xk8[:, :, :, m, :], kt8_d.ap()[:, :, :, m, :])
                nc.sync.dma_start(wk8[:, :, :, 512:1024], wk8_d.ap()[:, :, :, 512:1024])
                for m in (1, 3):
                    nc.sync.dma_start(xk8[:, :, :, m, :], kt8_d.ap()[:, :, :, m, :])
                for c in range(4):
                    nc.sync.dma_start(wvb[:, c, :], wvb_d.ap()[:, c, :])
                    nc.sync.dma_start(xkb[:, c, :, :], ktb_d.ap()[:, c, :, :])
                nc.sync.dma_start(wq8, wq8_d.ap())
                for m in (0, 2, 1, 3):
                    nc.sync.dma_start(xq8[:, :, :, m, :], qt8_d.ap()[:, :, :, m, :])
                nc.sync.dma_start(id8, id8_d.ap())
                nc.sync.dma_start(mb8, mb8_d.ap())
                nc.sync.dma_start(wob, wob_d.ap())

                def proj_pair(pool, hp8, s, w_sb, x_sb, dst):
                    """One head-pair, one m-parity: psum [128=(h2,d), mp, n];
                    single full-partition evacuation into dst[:, s, :, :]"""
                    pq = pool.tile([128, 2, N], F32, name="pq", tag="pq")
                    for mp in range(2):
                        m = 2 * mp + s
                        for c in range(2):
                            nc.tensor.matmul(
                                pq[:, mp, :],
                                w_sb[:, c, :, hp8 * 128 : (hp8 + 1) * 128],
                                x_sb[:, c, :, m, :],
                                start=(c == 0),
                                stop=(c == 1),
                                perf_mode=DR,
                            )
                    ecopy(pick2(1038, 1192), dst[:, s, :, :], pq)

                def p3_unit(u, pool, eng=None):
                    mi, kc, dvh = u
                    off = mi * 64
                    pv = pool.tile([128, 512], F32, name="pv", tag="pv")
                    for c in range(4):
                        nc.tensor.matmul(
                            pv,
                            xkb[:, c, mi, kc * 128 : (kc + 1) * 128],
                            wvb[:, c, dvh * 512 : (dvh + 1) * 512],
                            start=(c == 0),
                            stop=(c == 3),
                        )
                    v4 = vp2[kc].rearrange("p (h c) -> p h c", h=H)
                    ecopy(
                        eng if eng is not None else pick2(612, 658),
                        v4[:, dvh * 8 : (dvh + 1) * 8, off : off + 64],
                        pv.rearrange("p (h d) -> p h d", h=8),
                    )

                p3_units = [
                    (mi, kc, dvh)
                    for dvh in range(2)
                    for kc in range(KC)
                    for mi in range(M)
                ]

                with (
                    tc.tile_pool(name="ppj", bufs=3, space="PSUM") as ppj,
                    tc.tile_pool(name="ppv", bufs=2, space="PSUM") as ppv1,
                ):
                    for s in range(2):
                        for hp8 in range(HP):
                            proj_pair(ppj, hp8, s, wk8, xk8, kk[hp8])
                    # P1 with the 16 dvh=0 P3 units woven in
                    for hp8 in range(HP):
                        proj_pair(ppj, hp8, 0, wq8, xq8, qq[hp8])
                        p3_unit(p3_units[2 * hp8], ppv1)
                        proj_pair(ppj, hp8, 1, wq8, xq8, qq[hp8])
                        p3_unit(p3_units[2 * hp8 + 1], ppv1)

            # ============ attention (software-pipelined head loop) ============
            with (
                tc.tile_pool(name="exp", bufs=10) as expp,
                tc.tile_pool(name="sump", bufs=4) as sump,
                tc.tile_pool(name="repp", bufs=5) as repp,
                tc.tile_pool(name="opop", bufs=5) as opop,
                tc.tile_pool(name="pse", bufs=2, space="PSUM") as pse,
                tc.tile_pool(name="pso", bufs=1, space="PSUM") as pso,
                tc.tile_pool(name="ppv2", bufs=2, space="PSUM") as ppv2,
            ):

                def emit_e(h):
                    """E DR matmuls (two 64-partition s-passes) + mask-bias
                    matmul + exp -> masked em"""
                    g, h2 = h // 2, h % 2
                    sl = slice(h2 * 64, (h2 + 1) * 64)
                    em = []
                    for p in range(2):
                        pe2 = pse.tile([128, 2, N], F32, name="pe", tag="pe")
                        for half in range(2):
                            kc = 2 * p + half
                            for s in range(2):
                                nc.tensor.matmul(
                                    pe2[:, half, :],
                                    kk[g][sl, s, :, kc * 128 : (kc + 1) * 128],
                                    qq[g][sl, s, :, :],
                                    start=(s == 0),
                                    stop=False,
                                    perf_mode=DR,
                                )
                            nc.tensor.matmul(
                                pe2[:, half, :],
                                id8,
                                mb8[:, :, kc, :],
                                start=False,
                                stop=True,
                                perf_mode=DR,
                            )
                        ex2 = expp.tile([128, 2, N], BF, name="ex", tag="ex")
                        nc.scalar.activation(
                            ex2.rearrange("p t n -> p (t n)"),
                            pe2.rearrange("p t n -> p (t n)"),
                            AF.Exp,
                            scale=ESCALE,
                        )
                        em.append(ex2)
                    return em

                def emit_sums(h, em):
                    """bf16 tree-adds (DVE) + Pool all-reduce"""
                    t01 = sump.tile([128, 2, N], BF, name="t01", tag="t01")
                    nc.vector.tensor_add(t01, em[0], em[1])
                    s_all = sump.tile([128, N], BF, name="s_all", tag="s_all")
                    nc.vector.tensor_add(s_all, t01[:, 0, :], t01[:, 1, :])
                    s_red = sump.tile([128, N], BF, name="s_red", tag="s_red")
                    nc.gpsimd.partition_all_reduce(
                        s_red, s_all, channels=128, reduce_op=bass_isa.ReduceOp.add
                    )
                    return s_red

                def emit_recip(s_red):
                    rep = repp.tile([128, N], BF, name="rep", tag="rep")
                    with nc.allow_low_precision(reason="softmax 1/sum"):
                        nc.vector.reciprocal(rep, s_red)
                    return rep

                def emit_o(h, em):
                    """O matmuls into a 2-bank psum; evac alternates ACT/DVE"""
                    po2 = pso.tile([128, 2, N], F32, name="po", tag="po")
                    for mp in range(2):
                        for kc in range(KC):
                            nc.tensor.matmul(
                                po2[:, mp, :],
                                vp2[kc][:, h * 256 + mp * 128 : h * 256 + (mp + 1) * 128],
                                em[kc // 2][:, kc % 2, :],
                                start=(kc == 0),
                                stop=(kc == KC - 1),
                            )
                    opo = opop.tile([128, 2, N], BF, name="opo", tag="opo")
                    ecopy(nc.scalar, opo.rearrange("p t n -> p (t n)"),
                          po2.rearrange("p t n -> p (t n)"))
                    return opo

                def emit_norm(h, opo, rep, dve_all=False):
                    """normalize O into OT tiles: 2 muls on DVE, 2 on Pool"""
                    hp, hs = h // 2, h % 2
                    for i, (mp, s) in enumerate(
                        ((0, 0), (0, 1), (1, 0), (1, 1))
                    ):
                        eng = nc.vector if (dve_all or i % 2 == 0) else nc.gpsimd
                        eng.tensor_mul(
                            ot[hp][hs * 64 : (hs + 1) * 64, 2 * mp + s, :],
                            opo[s * 64 : (s + 1) * 64, mp, :],
                            rep[s * 64 : (s + 1) * 64, :],
                        )

                em_q, rep_q, opo_q = {}, {}, {}
                # dvh=1 P3 units woven ~1.5/head through heads 0..10 so all
                # are emitted before O(8) consumes their vp2 columns at
                # iteration 11. O lags E by 3 heads so the last O-chains
                # (and their evac/norm engine work) ride the P7 warm stretch.
                p3i = 16
                for h in range(H):
                    if h >= 3:
                        opo_q[h - 3] = emit_o(h - 3, em_q.pop(h - 3))
                    em_q[h] = emit_e(h)
                    while p3i < min(32, 16 + (16 * (h + 1) + 10) // 11):
                        p3_unit(p3_units[p3i], ppv2,
                                eng=nc.scalar if p3i % 2 == 0 else nc.vector)
                        p3i += 1
                    if h >= 1:
                        rep_q[h - 1] = emit_recip(emit_sums(h - 1, em_q[h - 1]))
                    if h >= 4:
                        emit_norm(h - 4, opo_q.pop(h - 4), rep_q.pop(h - 4))
                # epilogue: O(13..15) + norms 12..15 overlap the P7 warm units
                rep_q[15] = emit_recip(emit_sums(15, em_q[15]))
                opo_q[13] = emit_o(13, em_q.pop(13))
                emit_norm(12, opo_q.pop(12), rep_q.pop(12))
                opo_q[14] = emit_o(14, em_q.pop(14))
                emit_norm(13, opo_q.pop(13), rep_q.pop(13))
                opo_q[15] = emit_o(15, em_q.pop(15))
                emit_norm(14, opo_q.pop(14), rep_q.pop(14))
                emit_norm(15, opo_q.pop(15), rep_q.pop(15), dve_all=True)

            # ============ P7: output projection (bf16) ============
            # The first four psum groups run contraction chunks hp=0..5
            # before any hp>=6 step, so the PE keeps streaming while the
            # last two heads' normalized OT tiles are still being written.
            with (
                tc.tile_pool(name="outst", bufs=4) as outstp,
                tc.tile_pool(name="psf", bufs=4, space="PSUM") as psf,
            ):
                units = [(mi, ncc) for mi in range(M) for ncc in range(NC)]

                def p7_mms(pf, mi, ncc, dvh, hps, first, last):
                    for hp in hps:
                        nc.tensor.matmul(
                            pf,
                            ot[hp][:, mi, ncc * 128 : (ncc + 1) * 128],
                            wob[:, hp, dvh * 512 : (dvh + 1) * 512],
                            start=(hp == first),
                            stop=(hp == last),
                        )

                def p7_finish(u, pf0, pf1):
                    mi, ncc = u
                    ost = outstp.tile([128, 2, 512], F32, name="ost", tag="ost")
                    ecopy(pick2(612, 658), ost[:, 0, :], pf0)
                    ecopy(pick2(612, 658), ost[:, 1, :], pf1)
                    nc.sync.dma_start(
                        out_d.ap()[ncc * 128 : (ncc + 1) * 128, mi, :],
                        ost.rearrange("p a b -> p (a b)"),
                    )

                # warm stretch: 4 psum groups of hp0..5 for the first 2 units
                warm = []
                for u in units[:4]:
                    mi, ncc = u
                    pfs = []
                    for dvh in range(2):
                        pf = psf.tile([128, 512], F32, name="pf", tag="pf")
                        p7_mms(pf, mi, ncc, dvh, range(6), 0, HP - 1)
                        pfs.append(pf)
                    warm.append((u, pfs))
                for u, pfs in warm:
                    mi, ncc = u
                    for dvh in range(2):
                        p7_mms(pfs[dvh], mi, ncc, dvh, range(6, HP), 0, HP - 1)
                    p7_finish(u, *pfs)
                for u in units[4:-2]:
                    mi, ncc = u
                    pfs = []
                    for dvh in range(2):
                        pf = psf.tile([128, 512], F32, name="pf", tag="pf")
                        p7_mms(pf, mi, ncc, dvh, range(HP), 0, HP - 1)
                        pfs.append(pf)
                    p7_finish(u, *pfs)
                # last two units: per-half evac/DMA pipeline so the final
                # evacuation and store overlap the preceding unit's matmuls;
                # the very last half is split in two quarters so the final
                # serialized evac+DMA chain is as short as possible
                for u in units[-2:]:
                    last = u == units[-1]
                    mi, ncc = u
                    for dvh in range(2):
                        pf = psf.tile([128, 512], F32, name="pf", tag="pf")
                        p7_mms(pf, mi, ncc, dvh, range(HP), 0, HP - 1)
                        pieces = (
                            ((0, 384), (384, 512))
                            if (last and dvh == 1)
                            else ((0, 512),)
                        )
                        for lo, hi in pieces:
                            osh = outstp.tile([128, 512], F32, name="osh", tag="osh")
                            eng = nc.scalar if lo == 0 else nc.vector
                            ecopy(eng, osh[:, 0 : hi - lo], pf[:, lo:hi])
                            nc.sync.dma_start(
                                out_d.ap()[
                                    ncc * 128 : (ncc + 1) * 128,
                                    mi,
                                    dvh * 512 + lo : dvh * 512 + hi,
                                ],
                                osh[:, 0 : hi - lo],
                            )

    nc.compile()
    return nc


_NC_CACHE = None


def _get_nc():
    global _NC_CACHE
    if _NC_CACHE is None:
        _NC_CACHE = build_nc()
    return _NC_CACHE


def _dr_pack(a):
    """[512, ...] rows dq = c*256 + i*128 + p -> [128 = p, 2 = c, 2 = i, ...]"""
    s = a.shape[1:]
    return a.reshape(2, 2, 128, *s).transpose(2, 0, 1, *range(3, 3 + len(s)))


def kernel(Q, K, mask, w_q, w_k, w_v, w_o):
    from concourse.bass_utils import run_bass_kernel_spmd

    Q = np.asarray(Q, dtype=np.float32)
    K = np.asarray(K, dtype=np.float32)
    mask = np.asarray(mask)
    w_q = np.asarray(w_q, dtype=np.float32)
    w_k = np.asarray(w_k, dtype=np.float32)
    w_v = np.asarray(w_v, dtype=np.float32)
    w_o = np.asarray(w_o, dtype=np.float32)

    wq8 = np.ascontiguousarray(_dr_pack(w_q * WS)).astype(NPF8)
    wk8 = np.ascontiguousarray(_dr_pack(w_k * WS)).astype(NPF8)
    wvb = np.ascontiguousarray(
        w_v.reshape(4, 128, DV).transpose(1, 0, 2)
    ).astype(NPBF)
    wob = np.ascontiguousarray(
        w_o.reshape(HP, 128, DV).transpose(1, 0, 2)
    ).astype(NPBF)
    id8 = np.zeros((128, 2, 128), NPF8E5)
    id8[:, 0, :] = (np.eye(128) * 1024.0).astype(NPF8E5)

    in_maps = []
    for b in range(B):
        qt = np.ascontiguousarray(Q[b].transpose(2, 1, 0))   # [DQ, M, N]
        kt = np.ascontiguousarray(K[b].transpose(2, 1, 0))   # [DK, M, KN]
        mb = np.zeros((128, 2, KC, N), NPF8E5)
        # mask[b] is [N, KN]; mb[p, 0, kc, n] = MBIAS where mask[n, kc*128+p]==0
        mt = mask[b].T.reshape(KC, 128, N).transpose(1, 0, 2)  # [p, kc, n]
        mb[:, 0, :, :] = (MBIAS * (1 - mt)).astype(NPF8E5)
        in_maps.append(
            {
                "QT8": np.ascontiguousarray(_dr_pack(qt)).astype(NPF8),
                "KT8": np.ascontiguousarray(_dr_pack(kt)).astype(NPF8),
                "KTB": np.ascontiguousarray(
                    kt.reshape(4, 128, M, KN).transpose(1, 0, 2, 3)
                ).astype(NPBF),
                "MB8": mb,
                "ID8": id8,
                "WQ8": wq8,
                "WK8": wk8,
                "WVB": wvb,
                "WOB": wob,
            }
        )

    nc = _get_nc()
    r = run_bass_kernel_spmd(nc, in_maps, core_ids=list(range(B)), trace=False)
    return np.stack([r.results[b]["out"] for b in range(B)], axis=0)


if __name__ == "__main__":
    rng = np.random.default_rng(0)
    inputs = {
        "Q": rng.standard_normal((B, N, M, DQ), dtype=np.float32),
        "K": rng.standard_normal((B, KN, M, DK), dtype=np.float32),
        "mask": rng.integers(0, 2, (B, N, KN)).astype(np.int32),
        "w_q": (rng.standard_normal((DQ, DV), dtype=np.float32) * 0.02),
        "w_k": (rng.standard_normal((DK, DV), dtype=np.float32) * 0.02),
        "w_v": (rng.standard_normal((DK, DV), dtype=np.float32) * 0.02),
        "w_o": (rng.standard_normal((DV, DV), dtype=np.float32) * 0.02),
    }
    out = kernel(**inputs)
    print("out", out.shape, out.dtype, float(np.abs(out).max()))
